# revision 69
# baseline (speedup 1.0000x reference)
"""Trainium2 Bass kernel for nn_MABSINK (multi-head attention w/ 1-step Sinkhorn,
residuals, LayerNorms, fused output MLP).

Sharding: tensor-parallel over heads (8 heads -> 8 cores) for projections +
attention (dispatch 1); column-parallel over (batch, n-half) for LN0 + fc_o +
LN1 in the TRANSPOSED orientation (dispatch 2) so no on-chip transposes are
needed anywhere in dispatch 2 (d1's outputs are already [d, n]).

Both dispatches are software-pipelined: per-engine program order is interleaved
across batch/column tiles so the PE never sits behind a vector-chain of the
previous tile.

d1 math (eps=1, mu=nu):
  E = exp(S^T) stored [m, n] fp8;  omega*R[n] broadcast to all PSUM partitions
  via an all-ones fp8 DoubleRow matmul;  abc = 1/(omega*R) (fast reciprocal)
  ETa = E*abc in-place (accum_out -> cw);  vv = v/(omega*cw)
  attn^T via fp8 DoubleRow matmul;  o^T = MU*omega*attn + q^T (f32 residual)
d2: LN stats via broadcast all-ones bf16 matmuls (PE), apply on DVE/Scalar,
fc_o with transposed Wo chunks, fused bias+relu in PSUM evacuation.
"""

import functools
import math

import ml_dtypes
import numpy as np

B, N, D, H, DH = 4, 1024, 1024, 8, 128
MU = 1.0 / N + 1e-8  # == nu
LN_EPS = 1e-5
SCALE = 1.0 / math.sqrt(D)  # 1/32
NCORES = 8
OMEGA = 1.0 / 128.0  # fp8-range shift: ETa = E/(omega*R) lands in [0.03, 0.7]

LAST_EXEC_NS = {"d1": None, "d2": None}


def _mk_nc():
    import concourse.bacc as bacc

    return bacc.Bacc(
        "TRN2",
        target_bir_lowering=False,
        debug=False,
        enable_asserts=False,
        num_devices=NCORES,
    )


@functools.cache
def _build_d1():
    """Dispatch 1: projections + sinkhorn attention for one head (= one core).

    out[b] = (q + attn)^T as [DH, N] f32 per batch."""
    import concourse.bass as bass  # noqa: F401
    import concourse.mybir as mybir
    import concourse.tile as tile

    f32 = mybir.dt.float32
    bf16 = mybir.dt.bfloat16
    f8 = mybir.dt.float8e4
    AF = mybir.ActivationFunctionType
    ALU = mybir.AluOpType
    DR = mybir.MatmulPerfMode.DoubleRow

    nc = _mk_nc()
    QT = nc.dram_tensor("QT", [B, D, N], bf16, kind="ExternalInput").ap()
    KT8 = nc.dram_tensor("KT8", [B, 8, 128, N], f8, kind="ExternalInput").ap()
    WQ = nc.dram_tensor("WQ", [128, 8, 128], bf16, kind="ExternalInput").ap()
    WK = nc.dram_tensor("WK", [128, 8, 128], f8, kind="ExternalInput").ap()
    WV = nc.dram_tensor("WV", [128, 8, 128], f8, kind="ExternalInput").ap()
    BQ = nc.dram_tensor("BQ", [128, 1], f32, kind="ExternalInput").ap()
    BK = nc.dram_tensor("BK", [128, 1], f32, kind="ExternalInput").ap()
    BV = nc.dram_tensor("BV", [128, 1], f32, kind="ExternalInput").ap()
    ONES8 = nc.dram_tensor("ONES8", [128, 2, 128], f8, kind="ExternalInput").ap()
    IDENTB = nc.dram_tensor("IDENTB", [128, 128], bf16, kind="ExternalInput").ap()
    OT = nc.dram_tensor("OT", [B, DH, N], f32, kind="ExternalOutput").ap()

    with tile.TileContext(nc) as tc:
        with (
            tc.tile_pool(name="const", bufs=1) as constp,
            tc.tile_pool(name="io", bufs=3) as iop,
            tc.tile_pool(name="kt", bufs=2) as ktp,
            tc.tile_pool(name="mid", bufs=2) as midp,
            tc.tile_pool(name="et", bufs=2) as etp,
            tc.tile_pool(name="ps_proj", bufs=2, space="PSUM") as ps_proj,
            tc.tile_pool(name="ps_st", bufs=2, space="PSUM") as ps_st,
            tc.tile_pool(name="ps_r", bufs=2, space="PSUM") as ps_r,
            tc.tile_pool(name="ps_ot", bufs=2, space="PSUM") as ps_ot,
        ):
            # first batch's q inputs go on the DMA queue before everything else
            # (q-proj starts immediately; k-proj only needs ktc at ~+10us)
            qtcs0 = []
            for kc in range(8):
                t = iop.tile([128, N], bf16, tag="qtc", name=f"qtc0_{kc}")
                qtcs0.append(t)
            nc.sync.dma_start(qtcs0[0], QT[0, 0:128, :])

            wq = constp.tile([128, 8, 128], bf16)
            wk = constp.tile([128, 8, 128], f8)
            wv = constp.tile([128, 8, 128], f8)
            bq = constp.tile([128, 1], f32)
            bk = constp.tile([128, 1], f32)
            bv = constp.tile([128, 1], f32)
            ones8 = constp.tile([128, 2, 128], f8)
            identb = constp.tile([128, 128], bf16)
            nc.sync.dma_start(wq, WQ)
            nc.sync.dma_start(bq, BQ)
            for kc in range(1, 8):
                nc.sync.dma_start(qtcs0[kc], QT[0, kc * 128 : (kc + 1) * 128, :])
            ktc0 = ktp.tile([128, 8, N], f8, tag="ktc", name="ktc0")
            nc.sync.dma_start(ktc0, KT8[0].rearrange("kc p n -> p kc n"))
            nc.sync.dma_start(wk, WK)
            nc.sync.dma_start(wv, WV)
            nc.sync.dma_start(bk, BK)
            nc.sync.dma_start(bv, BV)
            nc.sync.dma_start(ones8, ONES8)
            nc.sync.dma_start(identb, IDENTB)
            # dummy exp to preload the activation table while DMAs run
            scratch1 = constp.tile([128, 1], f32)
            nc.scalar.activation(scratch1, bq, AF.Exp)

            def phase_a(b):
                """PE-heavy: projections, S+exp, v-transpose, omega*R."""
                if b == 0:
                    ktc = ktc0
                else:
                    ktc = ktp.tile([128, 8, N], f8, tag="ktc", name=f"ktc{b}")
                    nc.sync.dma_start(ktc, KT8[b].rearrange("kc p n -> p kc n"))

                qps = [
                    ps_proj.tile([128, 512], f32, tag="proj", name=f"qps{b}_{i}")
                    for i in range(2)
                ]
                for kc in range(8):
                    if b == 0:
                        qtc = qtcs0[kc]
                    else:
                        qtc = iop.tile(
                            [128, N], bf16, tag="qtc", name=f"qtc{b}_{kc}"
                        )
                        nc.sync.dma_start(qtc, QT[b, kc * 128 : (kc + 1) * 128, :])
                    for hf in range(2):
                        nc.tensor.matmul(
                            qps[hf],
                            wq[:, kc, :],
                            qtc[:, hf * 512 : (hf + 1) * 512],
                            start=(kc == 0),
                            stop=(kc == 7),
                        )
                # double evacuation: f32 copy for the residual, bf16 for S
                qTf = midp.tile([128, N], f32, tag="qTf", name=f"qTf{b}")
                qT = midp.tile([128, N], bf16, tag="qT", name=f"qT{b}")
                for hf in range(2):
                    hs = slice(hf * 512, (hf + 1) * 512)
                    nc.scalar.activation(qT[:, hs], qps[hf], AF.Identity, bias=bq)
                for hf in range(2):
                    hs = slice(hf * 512, (hf + 1) * 512)
                    nc.scalar.activation(qTf[:, hs], qps[hf], AF.Identity, bias=bq)

                kps = [
                    ps_proj.tile([128, 512], f32, tag="proj", name=f"kps{b}_{i}")
                    for i in range(2)
                ]
                for t in range(4):
                    for hf in range(2):
                        nc.tensor.matmul(
                            kps[hf],
                            wk[:, 2 * t : 2 * t + 2, :],
                            ktc[:, 2 * t : 2 * t + 2, hf * 512 : (hf + 1) * 512],
                            start=(t == 0),
                            stop=(t == 3),
                            perf_mode=DR,
                        )
                kT = midp.tile([128, N], bf16, tag="kT", name=f"kT{b}")
                for hf in range(2):
                    nc.scalar.activation(
                        kT[:, hf * 512 : (hf + 1) * 512],
                        kps[hf],
                        AF.Identity,
                        bias=bk,
                        scale=1.0 / 1024.0,
                    )

                vps = [
                    ps_proj.tile([128, 512], f32, tag="proj", name=f"vps{b}_{i}")
                    for i in range(2)
                ]
                for t in range(4):
                    for hf in range(2):
                        nc.tensor.matmul(
                            vps[hf],
                            wv[:, 2 * t : 2 * t + 2, :],
                            ktc[:, 2 * t : 2 * t + 2, hf * 512 : (hf + 1) * 512],
                            start=(t == 0),
                            stop=(t == 3),
                            perf_mode=DR,
                        )
                vT = midp.tile([128, N], bf16, tag="vT", name=f"vT{b}")
                for hf in range(2):
                    nc.scalar.activation(
                        vT[:, hf * 512 : (hf + 1) * 512],
                        vps[hf],
                        AF.Identity,
                        bias=bv,
                        scale=1.0 / 32.0,
                    )
                ET = etp.tile([128, 8, N], f8, tag="ET", name=f"ET{b}")
                for mt in range(8):
                    for hf in range(2):
                        sps = ps_st.tile(
                            [128, 512], f32, tag="st", name=f"sps{b}_{mt}_{hf}"
                        )
                        nc.tensor.matmul(
                            sps,
                            kT[:, mt * 128 : (mt + 1) * 128],
                            qT[:, hf * 512 : (hf + 1) * 512],
                            start=True,
                            stop=True,
                        )
                        nc.scalar.activation(
                            ET[:, mt, hf * 512 : (hf + 1) * 512], sps, AF.Exp
                        )

                # v = vT^T via PE transposes (hidden under the exp shadow)
                v = midp.tile([128, 8, 128], bf16, tag="v", name=f"v{b}")
                for mt in range(8):
                    tp = ps_st.tile([128, 128], bf16, tag="st", name=f"tp{b}_{mt}")
                    nc.tensor.transpose(
                        tp, vT[:, mt * 128 : (mt + 1) * 128], identb
                    )
                    nc.vector.tensor_copy(v[:, mt, :], tp)

                # omega*R broadcast to every PSUM partition (all-ones DR mm);
                # the reciprocal is deferred to phase_b_dve
                rpss = []
                for hf in range(2):
                    rps = ps_r.tile([128, 512], f32, tag="r", name=f"rps{b}_{hf}")
                    for t in range(4):
                        nc.tensor.matmul(
                            rps,
                            ones8,
                            ET[:, 2 * t : 2 * t + 2, hf * 512 : (hf + 1) * 512],
                            start=(t == 0),
                            stop=(t == 3),
                            perf_mode=DR,
                        )
                    rpss.append(rps)
                return dict(qTf=qTf, qT=qT, rpss=rpss, ET=ET, v=v)

            def phase_b(s, b):
                """Per-mt interleaved: ETa (+c accum) -> vv -> attn pair, then
                residual + output DMA."""
                ET, v, qTf = s["ET"], s["v"], s["qTf"]
                abcf = midp.tile([128, N], f32, tag="abcf", name=f"abcf{b}")
                for hf in range(2):
                    nc.vector.reciprocal_approx_fast(
                        abcf[:, hf * 512 : (hf + 1) * 512], s["rpss"][hf]
                    )
                # bf16 copy: 16-bit in1 makes the fp8 STT pass cheaper on DVE
                abc = midp.tile([128, N], bf16, tag="abc", name=f"abc{b}")
                nc.scalar.activation(abc, abcf, AF.Copy)
                cw = midp.tile([128, 8], f32, tag="cw", name=f"cw{b}")
                wrec = midp.tile([128, 8], f32, tag="wrec", name=f"wrec{b}")
                vv = midp.tile([128, 8, 128], f8, tag="vv", name=f"vv{b}")
                ops_ = [
                    ps_ot.tile([128, 512], f32, tag="ot", name=f"ot{b}_{i}")
                    for i in range(2)
                ]
                for t in range(4):
                    for j in range(2):
                        mt = 2 * t + j
                        nc.vector.scalar_tensor_tensor(
                            out=ET[:, mt, :],
                            in0=ET[:, mt, :],
                            scalar=1.0,
                            in1=abc,
                            op0=ALU.mult,
                            op1=ALU.mult,
                            accum_out=cw[:, mt : mt + 1],
                        )
                        nc.vector.reciprocal_approx_fast(
                            wrec[:, mt : mt + 1], cw[:, mt : mt + 1]
                        )
                        nc.vector.tensor_scalar(
                            out=vv[:, mt, :],
                            in0=v[:, mt, :],
                            scalar1=wrec[:, mt : mt + 1],
                            scalar2=1.0 / OMEGA,
                            op0=ALU.mult,
                            op1=ALU.mult,
                        )
                    for hf in range(2):
                        nc.tensor.matmul(
                            ops_[hf],
                            vv[:, 2 * t : 2 * t + 2, :],
                            ET[:, 2 * t : 2 * t + 2, hf * 512 : (hf + 1) * 512],
                            start=(t == 0),
                            stop=(t == 3),
                            perf_mode=DR,
                        )
                ofin = midp.tile([128, N], f32, tag="ofin", name=f"ofin{b}")
                for hf in range(2):
                    hs = slice(hf * 512, (hf + 1) * 512)
                    nc.vector.scalar_tensor_tensor(
                        out=ofin[:, hs],
                        in0=ops_[hf],
                        scalar=float(MU) * OMEGA,
                        in1=qTf[:, hs],
                        op0=ALU.mult,
                        op1=ALU.add,
                    )
                nc.sync.dma_start(OT[b], ofin)

            # pipeline: A0 A1 B0 A2 B1 A3 B2 B3
            state = []
            for b in range(B):
                state.append(phase_a(b))
                if b >= 1:
                    phase_b(state[b - 1], b - 1)
            phase_b(state[B - 1], B - 1)

    nc.compile()
    return nc


@functools.cache
def _build_d2():
    """Dispatch 2 (transposed): LN0 -> fc_o(+relu, residual) -> LN1 on a
    [1024 d, 512 n] column slab per core. LN stats via broadcast all-ones
    matmuls; no on-chip transposes."""
    import concourse.mybir as mybir
    import concourse.tile as tile

    f32 = mybir.dt.float32
    bf16 = mybir.dt.bfloat16
    AF = mybir.ActivationFunctionType
    ALU = mybir.AluOpType

    NC_ = 512  # full column slab; fc_o inner stages are pipelined per do-chunk

    nc = _mk_nc()
    XIN = nc.dram_tensor("XIN", [128, 8, 512], bf16, kind="ExternalInput").ap()
    WOT = nc.dram_tensor("WOT", [128, 8, D], bf16, kind="ExternalInput").ap()
    ONESB = nc.dram_tensor("ONESB", [128, 128], bf16, kind="ExternalInput").ap()
    BO = nc.dram_tensor("BO", [128, 8], f32, kind="ExternalInput").ap()
    G0 = nc.dram_tensor("G0", [128, 8], f32, kind="ExternalInput").ap()
    BE0 = nc.dram_tensor("BE0", [128, 8], f32, kind="ExternalInput").ap()
    G1 = nc.dram_tensor("G1", [128, 8], f32, kind="ExternalInput").ap()
    BE1 = nc.dram_tensor("BE1", [128, 8], f32, kind="ExternalInput").ap()
    EPSC = nc.dram_tensor("EPSC", [128, 1], f32, kind="ExternalInput").ap()
    OUT2 = nc.dram_tensor("OUT2", [128, 8, 512], f32, kind="ExternalOutput").ap()

    with tile.TileContext(nc) as tc:
        with (
            tc.tile_pool(name="const", bufs=1) as constp,
            tc.tile_pool(name="work", bufs=1) as wp,
            tc.tile_pool(name="small", bufs=2) as sp,
            tc.tile_pool(name="ps_mm", bufs=2, space="PSUM") as ps_mm,
            tc.tile_pool(name="ps_s", bufs=2, space="PSUM") as ps_s,
            tc.tile_pool(name="ps_l1", bufs=2, space="PSUM") as ps_l1,
        ):
            wot = constp.tile([128, 8, D], bf16)
            onesb = constp.tile([128, 128], bf16)
            boc = constp.tile([128, 8], f32)
            g0c = constp.tile([128, 8], f32)
            be0c = constp.tile([128, 8], f32)
            g1c = constp.tile([128, 8], f32)
            be1c = constp.tile([128, 8], f32)
            epsc = constp.tile([128, 1], f32)
            x = wp.tile([128, 8, NC_], bf16)
            nc.sync.dma_start(x, XIN)
            nc.sync.dma_start(onesb, ONESB)
            nc.sync.dma_start(epsc, EPSC)
            nc.sync.dma_start(wot, WOT)
            nc.sync.dma_start(boc, BO)
            nc.sync.dma_start(g0c, G0)
            nc.sync.dma_start(be0c, BE0)
            nc.sync.dma_start(g1c, G1)
            nc.sync.dma_start(be1c, BE1)

            def ln_stats(x_in, sq_in, uid):
                """Broadcast mean/rstd [128, NC_] from psum-accumulated
                column sums. Short chain: var folded into the Sqrt activation
                (scale=1/D, bias=eps), mean cast fused with its 1/D scale."""
                ps1, ps2 = sq_in
                mb16 = sp.tile([128, 1, NC_], bf16, tag="mb16", name=f"mb16_{uid}")
                nc.scalar.activation(
                    mb16[:, 0, :], ps1, AF.Copy, scale=1.0 / D
                )
                v1 = sp.tile([128, NC_], f32, tag="v1", name=f"v1_{uid}")
                nc.vector.scalar_tensor_tensor(
                    out=v1,
                    in0=ps1,
                    scalar=1.0,
                    in1=mb16[:, 0, :],
                    op0=ALU.mult,
                    op1=ALU.mult,
                )
                dif = sp.tile([128, NC_], f32, tag="dif", name=f"dif_{uid}")
                nc.vector.tensor_tensor(dif, ps2, v1, ALU.subtract)
                sq = sp.tile([128, NC_], f32, tag="sq", name=f"sq_{uid}")
                nc.scalar.activation(
                    sq, dif, AF.Sqrt, scale=1.0 / D, bias=epsc
                )
                rstd = sp.tile([128, NC_], f32, tag="rstd", name=f"rstd_{uid}")
                nc.vector.reciprocal_approx_fast(rstd, sq)
                rb16 = sp.tile([128, 1, NC_], bf16, tag="rb16", name=f"rb16_{uid}")
                nc.scalar.activation(rb16[:, 0, :], rstd, AF.Copy)
                return mb16, rb16

            def ln_xhat(x_in, mb16, rb16, uid):
                """(x - mean) * rstd, per chunk (stride-0 broadcast DVE reads
                measured slower than per-chunk ops)."""
                xr = sp.tile([128, 8, NC_], bf16, tag="xra", name=f"xra_{uid}")
                for dc in range(8):
                    xc = sp.tile(
                        [128, NC_], bf16, tag="xca", name=f"xca_{uid}_{dc}"
                    )
                    nc.vector.tensor_tensor(
                        xc, x_in[:, dc, :], mb16[:, 0, :], ALU.subtract
                    )
                    nc.vector.tensor_tensor(
                        xr[:, dc, :], xc, rb16[:, 0, :], ALU.mult
                    )
                return xr

            # ---- LN0: stats then apply ----
            ps1 = ps_s.tile([128, NC_], f32, tag="s", name="ps1_ln0")
            for dc in range(8):
                nc.tensor.matmul(
                    ps1, onesb, x[:, dc, :], start=(dc == 0), stop=(dc == 7)
                )
            xsq = sp.tile([128, 8, NC_], bf16, tag="xsq", name="xsq_ln0")
            nc.vector.tensor_tensor(xsq, x, x, ALU.mult)
            ps2 = ps_s.tile([128, NC_], f32, tag="s", name="ps2_ln0")
            for dc in range(8):
                nc.tensor.matmul(
                    ps2, onesb, xsq[:, dc, :], start=(dc == 0), stop=(dc == 7)
                )
            mb16, rb16 = ln_stats(x, (ps1, ps2), "ln0")
            xhat0 = ln_xhat(x, mb16, rb16, "ln0")
            oln = wp.tile([128, 8, NC_], bf16)
            for dc in range(8):
                nc.scalar.activation(
                    oln[:, dc, :],
                    xhat0[:, dc, :],
                    AF.Identity,
                    scale=g0c[:, dc : dc + 1],
                    bias=be0c[:, dc : dc + 1],
                )

            # ---- fc_o with fused relu/residual/LN1-stat accumulation ----
            t1 = wp.tile([128, 8, NC_], bf16)
            o2 = wp.tile([128, 8, NC_], bf16)
            o2sq = wp.tile([128, 8, NC_], bf16)
            l1s1 = ps_l1.tile([128, NC_], f32, tag="l1", name="l1s1")
            l1s2 = ps_l1.tile([128, NC_], f32, tag="l1", name="l1s2")
            def stat_mms(do):
                # LN1 stat contribution for chunk do (emitted one iteration
                # late so the PE never waits on the relu->residual chain)
                nc.tensor.matmul(
                    l1s1,
                    onesb,
                    o2[:, do, :],
                    start=(do == 0),
                    stop=(do == 7),
                    skip_group_check=True,
                )
                nc.tensor.matmul(
                    l1s2,
                    onesb,
                    o2sq[:, do, :],
                    start=(do == 0),
                    stop=(do == 7),
                    skip_group_check=True,
                )

            for do in range(8):
                ps = ps_mm.tile([128, NC_], f32, tag="mm", name=f"mm{do}")
                for di in range(8):
                    nc.tensor.matmul(
                        ps,
                        wot[:, di, do * 128 : (do + 1) * 128],
                        oln[:, di, :],
                        start=(di == 0),
                        stop=(di == 7),
                    )
                nc.scalar.activation(
                    t1[:, do, :], ps, AF.Relu, bias=boc[:, do : do + 1]
                )
                nc.vector.tensor_tensor(
                    o2[:, do, :], t1[:, do, :], oln[:, do, :], ALU.add
                )
                nc.vector.tensor_tensor(
                    o2sq[:, do, :], o2[:, do, :], o2[:, do, :], ALU.mult
                )
                if do >= 1:
                    stat_mms(do - 1)
            stat_mms(7)

            # ---- LN1 (chunk-wise output DMA overlaps the apply) ----
            mb16b, rb16b = ln_stats(o2, (l1s1, l1s2), "ln1")
            xhat1 = ln_xhat(o2, mb16b, rb16b, "ln1")
            o3 = wp.tile([128, 8, NC_], f32)
            for dc in range(8):
                nc.scalar.activation(
                    o3[:, dc, :],
                    xhat1[:, dc, :],
                    AF.Identity,
                    scale=g1c[:, dc : dc + 1],
                    bias=be1c[:, dc : dc + 1],
                )
                nc.sync.dma_start(OUT2[:, dc, :], o3[:, dc, :])

    nc.compile()
    return nc


def _run(nc, in_maps, trace=False):
    from concourse.bass_utils import run_bass_kernel_spmd

    return run_bass_kernel_spmd(nc, in_maps, list(range(NCORES)), trace=trace)


def kernel(**inputs):
    trace = bool(int(__import__("os").environ.get("KERNEL_TRACE", "0")))
    f32 = np.float32
    bf16 = ml_dtypes.bfloat16
    f8 = ml_dtypes.float8_e4m3fn
    Q = np.ascontiguousarray(inputs["Q"], dtype=f32)
    K = np.ascontiguousarray(inputs["K"], dtype=f32)
    Wq, Wk, Wv, Wo = (np.asarray(inputs[k], f32) for k in ("Wq", "Wk", "Wv", "Wo"))
    bq, bk, bv, bo = (np.asarray(inputs[k], f32) for k in ("bq", "bk", "bv", "bo"))
    g0, be0, g1, be1 = (np.asarray(inputs[k], f32) for k in ("g0", "be0", "g1", "be1"))

    QT = np.ascontiguousarray(Q.transpose(0, 2, 1)).astype(bf16)
    KT8 = (
        np.ascontiguousarray(K.transpose(0, 2, 1)).reshape(B, 8, 128, N).astype(f8)
    )
    ones8 = np.full((128, 2, 128), OMEGA, dtype=f8)
    identb = np.eye(128, dtype=bf16)

    in_maps = []
    for h in range(H):
        hs = slice(h * DH, (h + 1) * DH)
        wqh = np.ascontiguousarray(
            Wq[:, hs].reshape(8, 128, 128).transpose(1, 0, 2)
        ).astype(bf16)
        wkh = np.ascontiguousarray(
            (Wk[:, hs] * 32.0).reshape(8, 128, 128).transpose(1, 0, 2)
        ).astype(f8)
        wvh = np.ascontiguousarray(
            (Wv[:, hs] * 32.0).reshape(8, 128, 128).transpose(1, 0, 2)
        ).astype(f8)
        in_maps.append(
            {
                "QT": QT,
                "KT8": KT8,
                "WQ": wqh,
                "WK": wkh,
                "WV": wvh,
                "BQ": bq[hs].reshape(128, 1).astype(f32),
                "BK": (bk[hs] * SCALE).reshape(128, 1).astype(f32),
                "BV": bv[hs].reshape(128, 1).astype(f32),
                "ONES8": ones8,
                "IDENTB": identb,
            }
        )

    r1 = _run(_build_d1(), in_maps, trace=trace)
    LAST_EXEC_NS["d1"] = r1.exec_time_ns

    # ---- host reshard: per-head [B, DH, N] f32 -> per-core [8, 128, 512] ----
    # core c <-> (batch c//2, n-half c%2); chunk dim = head index
    OTall = np.stack([r1.results[h]["OT"] for h in range(H)])  # [8, B, 128, N]
    wot_in = np.ascontiguousarray(
        Wo.reshape(8, 128, D).transpose(1, 0, 2)
    ).astype(bf16)
    onesb = np.ones((128, 128), dtype=bf16)
    col = lambda z: np.ascontiguousarray(z.reshape(8, 128).T, dtype=f32)
    boc, g0c, be0c, g1c, be1c = col(bo), col(g0), col(be0), col(g1), col(be1)
    in_maps2 = []
    for c in range(NCORES):
        b, nh = c // 2, c % 2
        # partition-major [128 p, 8 dc, 512 n] so the device DMA is contiguous
        xin = np.ascontiguousarray(
            OTall[:, b, :, nh * 512 : (nh + 1) * 512].transpose(1, 0, 2)
        ).astype(bf16)
        in_maps2.append(
            {
                "XIN": xin,
                "WOT": wot_in,
                "ONESB": onesb,
                "BO": boc,
                "G0": g0c,
                "BE0": be0c,
                "G1": g1c,
                "BE1": be1c,
                "EPSC": np.full((128, 1), LN_EPS, dtype=f32),
            }
        )
    r2 = _run(_build_d2(), in_maps2, trace=trace)
    LAST_EXEC_NS["d2"] = r2.exec_time_ns

    # ---- host unshard: [128 p, 8 dc, 512 n] -> [n, d] rows of O ----
    out = np.empty((B, N, D), dtype=f32)
    for c in range(NCORES):
        b, nh = c // 2, c % 2
        slab = r2.results[c]["OUT2"]  # [128, 8, 512] f32
        out[b, nh * 512 : (nh + 1) * 512, :] = (
            slab.transpose(2, 1, 0).reshape(512, D)
        )
    return out


# revision 71
# speedup vs baseline: 1.0652x; 1.0652x over previous
"""Trainium2 Bass kernel for nn_MABSINK (multi-head attention w/ 1-step Sinkhorn,
residuals, LayerNorms, fused output MLP).

Sharding: tensor-parallel over heads (8 heads -> 8 cores) for projections +
attention (dispatch 1); column-parallel over (batch, n-half) for LN0 + fc_o +
LN1 in the TRANSPOSED orientation (dispatch 2) so no on-chip transposes are
needed anywhere in dispatch 2 (d1's outputs are already [d, n]).

Both dispatches are software-pipelined: per-engine program order is interleaved
across batch/column tiles so the PE never sits behind a vector-chain of the
previous tile.

d1 math (eps=1, mu=nu):
  E = exp(S^T) stored [m, n] fp8;  omega*R[n] broadcast to all PSUM partitions
  via an all-ones fp8 DoubleRow matmul;  abc = 1/(omega*R) (fast reciprocal)
  ETa = E*abc in-place (accum_out -> cw);  vv = v/(omega*cw)
  attn^T via fp8 DoubleRow matmul;  o^T = MU*omega*attn + q^T (f32 residual)
d2: LN stats via broadcast all-ones bf16 matmuls (PE), apply on DVE/Scalar,
fc_o with transposed Wo chunks, fused bias+relu in PSUM evacuation.
"""

import functools
import math

import ml_dtypes
import numpy as np

B, N, D, H, DH = 4, 1024, 1024, 8, 128
MU = 1.0 / N + 1e-8  # == nu
LN_EPS = 1e-5
SCALE = 1.0 / math.sqrt(D)  # 1/32
NCORES = 8
OMEGA = 1.0 / 128.0  # fp8-range shift: ETa = E/(omega*R) lands in [0.03, 0.7]

LAST_EXEC_NS = {"d1": None, "d2": None}


def _mk_nc():
    import concourse.bacc as bacc

    return bacc.Bacc(
        "TRN2",
        target_bir_lowering=False,
        debug=False,
        enable_asserts=False,
        num_devices=NCORES,
    )


@functools.cache
def _build_d1():
    """Dispatch 1: projections + sinkhorn attention for one head (= one core).

    out[b] = (q + attn)^T as [DH, N] f32 per batch."""
    import concourse.bass as bass  # noqa: F401
    import concourse.mybir as mybir
    import concourse.tile as tile

    f32 = mybir.dt.float32
    bf16 = mybir.dt.bfloat16
    f8 = mybir.dt.float8e4
    AF = mybir.ActivationFunctionType
    ALU = mybir.AluOpType
    DR = mybir.MatmulPerfMode.DoubleRow

    nc = _mk_nc()
    QT = nc.dram_tensor("QT", [B, D, N], bf16, kind="ExternalInput").ap()
    KT8 = nc.dram_tensor("KT8", [B, 8, 128, N], f8, kind="ExternalInput").ap()
    WQ = nc.dram_tensor("WQ", [128, 8, 128], bf16, kind="ExternalInput").ap()
    WK = nc.dram_tensor("WK", [128, 8, 128], f8, kind="ExternalInput").ap()
    WV = nc.dram_tensor("WV", [128, 8, 128], f8, kind="ExternalInput").ap()
    BQ = nc.dram_tensor("BQ", [128, 1], f32, kind="ExternalInput").ap()
    BK = nc.dram_tensor("BK", [128, 1], f32, kind="ExternalInput").ap()
    BV = nc.dram_tensor("BV", [128, 1], f32, kind="ExternalInput").ap()
    ONES8 = nc.dram_tensor("ONES8", [128, 2, 128], f8, kind="ExternalInput").ap()
    IDENTB = nc.dram_tensor("IDENTB", [128, 128], bf16, kind="ExternalInput").ap()
    OT = nc.dram_tensor("OT", [B, DH, N], f32, kind="ExternalOutput").ap()

    with tile.TileContext(nc) as tc:
        with (
            tc.tile_pool(name="const", bufs=1) as constp,
            tc.tile_pool(name="io", bufs=3) as iop,
            tc.tile_pool(name="kt", bufs=2) as ktp,
            tc.tile_pool(name="mid", bufs=2) as midp,
            tc.tile_pool(name="et", bufs=2) as etp,
            tc.tile_pool(name="ps_proj", bufs=2, space="PSUM") as ps_proj,
            tc.tile_pool(name="ps_st", bufs=2, space="PSUM") as ps_st,
            tc.tile_pool(name="ps_r", bufs=2, space="PSUM") as ps_r,
            tc.tile_pool(name="ps_ot", bufs=2, space="PSUM") as ps_ot,
        ):
            # first batch's q inputs go on the DMA queue before everything else
            # (q-proj starts immediately; k-proj only needs ktc at ~+10us)
            qtcs0 = []
            for kc in range(8):
                t = iop.tile([128, N], bf16, tag="qtc", name=f"qtc0_{kc}")
                qtcs0.append(t)
            nc.sync.dma_start(qtcs0[0], QT[0, 0:128, :])

            wq = constp.tile([128, 8, 128], bf16)
            wk = constp.tile([128, 8, 128], f8)
            wv = constp.tile([128, 8, 128], f8)
            bq = constp.tile([128, 1], f32)
            bk = constp.tile([128, 1], f32)
            bv = constp.tile([128, 1], f32)
            ones8 = constp.tile([128, 2, 128], f8)
            identb = constp.tile([128, 128], bf16)
            nc.sync.dma_start(wq, WQ)
            nc.sync.dma_start(bq, BQ)
            for kc in range(1, 8):
                nc.sync.dma_start(qtcs0[kc], QT[0, kc * 128 : (kc + 1) * 128, :])
            ktc0 = ktp.tile([128, 8, N], f8, tag="ktc", name="ktc0")
            nc.sync.dma_start(ktc0, KT8[0].rearrange("kc p n -> p kc n"))
            nc.sync.dma_start(wk, WK)
            nc.sync.dma_start(wv, WV)
            nc.sync.dma_start(bk, BK)
            nc.sync.dma_start(bv, BV)
            nc.sync.dma_start(ones8, ONES8)
            nc.sync.dma_start(identb, IDENTB)
            # dummy exp to preload the activation table while DMAs run
            scratch1 = constp.tile([128, 1], f32)
            nc.scalar.activation(scratch1, bq, AF.Exp)

            def phase_a(b):
                """PE-heavy: projections, S+exp, v-transpose, omega*R."""
                if b == 0:
                    ktc = ktc0
                else:
                    ktc = ktp.tile([128, 8, N], f8, tag="ktc", name=f"ktc{b}")
                    nc.sync.dma_start(ktc, KT8[b].rearrange("kc p n -> p kc n"))

                qps = [
                    ps_proj.tile([128, 512], f32, tag="proj", name=f"qps{b}_{i}")
                    for i in range(2)
                ]
                for kc in range(8):
                    if b == 0:
                        qtc = qtcs0[kc]
                    else:
                        qtc = iop.tile(
                            [128, N], bf16, tag="qtc", name=f"qtc{b}_{kc}"
                        )
                        nc.sync.dma_start(qtc, QT[b, kc * 128 : (kc + 1) * 128, :])
                    for hf in range(2):
                        nc.tensor.matmul(
                            qps[hf],
                            wq[:, kc, :],
                            qtc[:, hf * 512 : (hf + 1) * 512],
                            start=(kc == 0),
                            stop=(kc == 7),
                        )
                # single bf16 evacuation serves both S and the residual (the
                # bf16 q rounding is washed out by d2's bf16 input anyway)
                qT = midp.tile([128, N], bf16, tag="qT", name=f"qT{b}")
                for hf in range(2):
                    hs = slice(hf * 512, (hf + 1) * 512)
                    nc.scalar.activation(qT[:, hs], qps[hf], AF.Identity, bias=bq)

                kps = [
                    ps_proj.tile([128, 512], f32, tag="proj", name=f"kps{b}_{i}")
                    for i in range(2)
                ]
                for t in range(4):
                    for hf in range(2):
                        nc.tensor.matmul(
                            kps[hf],
                            wk[:, 2 * t : 2 * t + 2, :],
                            ktc[:, 2 * t : 2 * t + 2, hf * 512 : (hf + 1) * 512],
                            start=(t == 0),
                            stop=(t == 3),
                            perf_mode=DR,
                        )
                kT = midp.tile([128, N], bf16, tag="kT", name=f"kT{b}")
                for hf in range(2):
                    nc.scalar.activation(
                        kT[:, hf * 512 : (hf + 1) * 512],
                        kps[hf],
                        AF.Identity,
                        bias=bk,
                        scale=1.0 / 1024.0,
                    )

                vps = [
                    ps_proj.tile([128, 512], f32, tag="proj", name=f"vps{b}_{i}")
                    for i in range(2)
                ]
                for t in range(4):
                    for hf in range(2):
                        nc.tensor.matmul(
                            vps[hf],
                            wv[:, 2 * t : 2 * t + 2, :],
                            ktc[:, 2 * t : 2 * t + 2, hf * 512 : (hf + 1) * 512],
                            start=(t == 0),
                            stop=(t == 3),
                            perf_mode=DR,
                        )
                vT = midp.tile([128, N], bf16, tag="vT", name=f"vT{b}")
                for hf in range(2):
                    nc.scalar.activation(
                        vT[:, hf * 512 : (hf + 1) * 512],
                        vps[hf],
                        AF.Identity,
                        bias=bv,
                        scale=1.0 / 32.0,
                    )
                ET = etp.tile([128, 8, N], f8, tag="ET", name=f"ET{b}")
                for mt in range(8):
                    for hf in range(2):
                        sps = ps_st.tile(
                            [128, 512], f32, tag="st", name=f"sps{b}_{mt}_{hf}"
                        )
                        nc.tensor.matmul(
                            sps,
                            kT[:, mt * 128 : (mt + 1) * 128],
                            qT[:, hf * 512 : (hf + 1) * 512],
                            start=True,
                            stop=True,
                        )
                        nc.scalar.activation(
                            ET[:, mt, hf * 512 : (hf + 1) * 512], sps, AF.Exp
                        )

                # v = vT^T via PE transposes (hidden under the exp shadow)
                v = midp.tile([128, 8, 128], bf16, tag="v", name=f"v{b}")
                for mt in range(8):
                    tp = ps_st.tile([128, 128], bf16, tag="st", name=f"tp{b}_{mt}")
                    nc.tensor.transpose(
                        tp, vT[:, mt * 128 : (mt + 1) * 128], identb
                    )
                    nc.vector.tensor_copy(v[:, mt, :], tp)

                # omega*R broadcast to every PSUM partition (all-ones DR mm);
                # the reciprocal is deferred to phase_b_dve
                rpss = []
                for hf in range(2):
                    rps = ps_r.tile([128, 512], f32, tag="r", name=f"rps{b}_{hf}")
                    for t in range(4):
                        nc.tensor.matmul(
                            rps,
                            ones8,
                            ET[:, 2 * t : 2 * t + 2, hf * 512 : (hf + 1) * 512],
                            start=(t == 0),
                            stop=(t == 3),
                            perf_mode=DR,
                        )
                    rpss.append(rps)
                return dict(qT=qT, rpss=rpss, ET=ET, v=v)

            def phase_b(s, b):
                """Per-mt interleaved: ETa (+c accum) -> vv -> attn pair, then
                residual + output DMA."""
                ET, v, qT = s["ET"], s["v"], s["qT"]
                abcf = midp.tile([128, N], f32, tag="abcf", name=f"abcf{b}")
                for hf in range(2):
                    nc.vector.reciprocal_approx_fast(
                        abcf[:, hf * 512 : (hf + 1) * 512], s["rpss"][hf]
                    )
                # bf16 copy: 16-bit in1 makes the fp8 STT pass cheaper on DVE
                abc = midp.tile([128, N], bf16, tag="abc", name=f"abc{b}")
                nc.scalar.activation(abc, abcf, AF.Copy)
                cw = midp.tile([128, 8], f32, tag="cw", name=f"cw{b}")
                wrec = midp.tile([128, 8], f32, tag="wrec", name=f"wrec{b}")
                vv = midp.tile([128, 8, 128], f8, tag="vv", name=f"vv{b}")
                ops_ = [
                    ps_ot.tile([128, 512], f32, tag="ot", name=f"ot{b}_{i}")
                    for i in range(2)
                ]
                for t in range(4):
                    for j in range(2):
                        mt = 2 * t + j
                        nc.vector.scalar_tensor_tensor(
                            out=ET[:, mt, :],
                            in0=ET[:, mt, :],
                            scalar=1.0,
                            in1=abc,
                            op0=ALU.mult,
                            op1=ALU.mult,
                            accum_out=cw[:, mt : mt + 1],
                        )
                        nc.vector.reciprocal_approx_fast(
                            wrec[:, mt : mt + 1], cw[:, mt : mt + 1]
                        )
                        nc.vector.tensor_scalar(
                            out=vv[:, mt, :],
                            in0=v[:, mt, :],
                            scalar1=wrec[:, mt : mt + 1],
                            scalar2=1.0 / OMEGA,
                            op0=ALU.mult,
                            op1=ALU.mult,
                        )
                    for hf in range(2):
                        nc.tensor.matmul(
                            ops_[hf],
                            vv[:, 2 * t : 2 * t + 2, :],
                            ET[:, 2 * t : 2 * t + 2, hf * 512 : (hf + 1) * 512],
                            start=(t == 0),
                            stop=(t == 3),
                            perf_mode=DR,
                        )
                ofin = midp.tile([128, N], f32, tag="ofin", name=f"ofin{b}")
                for hf in range(2):
                    hs = slice(hf * 512, (hf + 1) * 512)
                    nc.vector.scalar_tensor_tensor(
                        out=ofin[:, hs],
                        in0=ops_[hf],
                        scalar=float(MU) * OMEGA,
                        in1=qT[:, hs],
                        op0=ALU.mult,
                        op1=ALU.add,
                    )
                nc.sync.dma_start(OT[b], ofin)

            # pipeline: A0 A1 B0 A2 B1 A3 B2 B3
            state = []
            for b in range(B):
                state.append(phase_a(b))
                if b >= 1:
                    phase_b(state[b - 1], b - 1)
            phase_b(state[B - 1], B - 1)

    nc.compile()
    return nc


@functools.cache
def _build_d2():
    """Dispatch 2 (transposed): LN0 -> fc_o(+relu, residual) -> LN1 on a
    [1024 d, 512 n] column slab per core. LN stats via broadcast all-ones
    matmuls; no on-chip transposes."""
    import concourse.mybir as mybir
    import concourse.tile as tile

    f32 = mybir.dt.float32
    bf16 = mybir.dt.bfloat16
    AF = mybir.ActivationFunctionType
    ALU = mybir.AluOpType

    NC_ = 512  # full column slab; fc_o inner stages are pipelined per do-chunk

    nc = _mk_nc()
    XIN = nc.dram_tensor("XIN", [128, 8, 512], bf16, kind="ExternalInput").ap()
    WOT = nc.dram_tensor("WOT", [128, 8, D], bf16, kind="ExternalInput").ap()
    ONESB = nc.dram_tensor("ONESB", [128, 128], bf16, kind="ExternalInput").ap()
    BO = nc.dram_tensor("BO", [128, 8], f32, kind="ExternalInput").ap()
    G0 = nc.dram_tensor("G0", [128, 8], f32, kind="ExternalInput").ap()
    BE0 = nc.dram_tensor("BE0", [128, 8], f32, kind="ExternalInput").ap()
    G1 = nc.dram_tensor("G1", [128, 8], f32, kind="ExternalInput").ap()
    BE1 = nc.dram_tensor("BE1", [128, 8], f32, kind="ExternalInput").ap()
    EPSC = nc.dram_tensor("EPSC", [128, 1], f32, kind="ExternalInput").ap()
    OUT2 = nc.dram_tensor("OUT2", [128, 8, 512], f32, kind="ExternalOutput").ap()

    with tile.TileContext(nc) as tc:
        with (
            tc.tile_pool(name="const", bufs=1) as constp,
            tc.tile_pool(name="work", bufs=1) as wp,
            tc.tile_pool(name="small", bufs=2) as sp,
            tc.tile_pool(name="ps_mm", bufs=2, space="PSUM") as ps_mm,
            tc.tile_pool(name="ps_s", bufs=2, space="PSUM") as ps_s,
            tc.tile_pool(name="ps_l1", bufs=2, space="PSUM") as ps_l1,
        ):
            wot = constp.tile([128, 8, D], bf16)
            onesb = constp.tile([128, 128], bf16)
            boc = constp.tile([128, 8], f32)
            g0c = constp.tile([128, 8], f32)
            be0c = constp.tile([128, 8], f32)
            g1c = constp.tile([128, 8], f32)
            be1c = constp.tile([128, 8], f32)
            epsc = constp.tile([128, 1], f32)
            x = wp.tile([128, 8, NC_], bf16)
            nc.sync.dma_start(x, XIN)
            nc.sync.dma_start(onesb, ONESB)
            nc.sync.dma_start(epsc, EPSC)
            nc.sync.dma_start(wot, WOT)
            nc.sync.dma_start(boc, BO)
            nc.sync.dma_start(g0c, G0)
            nc.sync.dma_start(be0c, BE0)
            nc.sync.dma_start(g1c, G1)
            nc.sync.dma_start(be1c, BE1)

            def ln_stats(x_in, sq_in, uid):
                """Broadcast mean/rstd [128, NC_] from psum-accumulated
                column sums. Short chain: var folded into the Sqrt activation
                (scale=1/D, bias=eps), mean cast fused with its 1/D scale."""
                ps1, ps2 = sq_in
                mb16 = sp.tile([128, 1, NC_], bf16, tag="mb16", name=f"mb16_{uid}")
                nc.scalar.activation(
                    mb16[:, 0, :], ps1, AF.Copy, scale=1.0 / D
                )
                v1 = sp.tile([128, NC_], f32, tag="v1", name=f"v1_{uid}")
                nc.vector.scalar_tensor_tensor(
                    out=v1,
                    in0=ps1,
                    scalar=1.0,
                    in1=mb16[:, 0, :],
                    op0=ALU.mult,
                    op1=ALU.mult,
                )
                dif = sp.tile([128, NC_], f32, tag="dif", name=f"dif_{uid}")
                nc.vector.tensor_tensor(dif, ps2, v1, ALU.subtract)
                sq = sp.tile([128, NC_], f32, tag="sq", name=f"sq_{uid}")
                nc.scalar.activation(
                    sq, dif, AF.Sqrt, scale=1.0 / D, bias=epsc
                )
                rstd = sp.tile([128, NC_], f32, tag="rstd", name=f"rstd_{uid}")
                nc.vector.reciprocal_approx_fast(rstd, sq)
                rb16 = sp.tile([128, 1, NC_], bf16, tag="rb16", name=f"rb16_{uid}")
                nc.scalar.activation(rb16[:, 0, :], rstd, AF.Copy)
                return mb16, rb16

            def ln_xhat(x_in, mb16, rb16, uid):
                """(x - mean) * rstd, per chunk (stride-0 broadcast DVE reads
                measured slower than per-chunk ops)."""
                xr = sp.tile([128, 8, NC_], bf16, tag="xra", name=f"xra_{uid}")
                for dc in range(8):
                    xc = sp.tile(
                        [128, NC_], bf16, tag="xca", name=f"xca_{uid}_{dc}"
                    )
                    nc.vector.tensor_tensor(
                        xc, x_in[:, dc, :], mb16[:, 0, :], ALU.subtract
                    )
                    nc.vector.tensor_tensor(
                        xr[:, dc, :], xc, rb16[:, 0, :], ALU.mult
                    )
                return xr

            # ---- LN0: stats then apply ----
            ps1 = ps_s.tile([128, NC_], f32, tag="s", name="ps1_ln0")
            for dc in range(8):
                nc.tensor.matmul(
                    ps1, onesb, x[:, dc, :], start=(dc == 0), stop=(dc == 7)
                )
            xsq = sp.tile([128, 8, NC_], bf16, tag="xsq", name="xsq_ln0")
            nc.vector.tensor_tensor(xsq, x, x, ALU.mult)
            ps2 = ps_s.tile([128, NC_], f32, tag="s", name="ps2_ln0")
            for dc in range(8):
                nc.tensor.matmul(
                    ps2, onesb, xsq[:, dc, :], start=(dc == 0), stop=(dc == 7)
                )
            mb16, rb16 = ln_stats(x, (ps1, ps2), "ln0")
            xhat0 = ln_xhat(x, mb16, rb16, "ln0")
            oln = wp.tile([128, 8, NC_], bf16)
            for dc in range(8):
                nc.scalar.activation(
                    oln[:, dc, :],
                    xhat0[:, dc, :],
                    AF.Identity,
                    scale=g0c[:, dc : dc + 1],
                    bias=be0c[:, dc : dc + 1],
                )

            # ---- fc_o with fused relu/residual/LN1-stat accumulation ----
            t1 = wp.tile([128, 8, NC_], bf16)
            o2 = wp.tile([128, 8, NC_], bf16)
            o2sq = wp.tile([128, 8, NC_], bf16)
            l1s1 = ps_l1.tile([128, NC_], f32, tag="l1", name="l1s1")
            l1s2 = ps_l1.tile([128, NC_], f32, tag="l1", name="l1s2")
            def stat_mms(do):
                # LN1 stat contribution for chunk do (emitted one iteration
                # late so the PE never waits on the relu->residual chain)
                nc.tensor.matmul(
                    l1s1,
                    onesb,
                    o2[:, do, :],
                    start=(do == 0),
                    stop=(do == 7),
                    skip_group_check=True,
                )
                nc.tensor.matmul(
                    l1s2,
                    onesb,
                    o2sq[:, do, :],
                    start=(do == 0),
                    stop=(do == 7),
                    skip_group_check=True,
                )

            for do in range(8):
                ps = ps_mm.tile([128, NC_], f32, tag="mm", name=f"mm{do}")
                for di in range(8):
                    nc.tensor.matmul(
                        ps,
                        wot[:, di, do * 128 : (do + 1) * 128],
                        oln[:, di, :],
                        start=(di == 0),
                        stop=(di == 7),
                    )
                nc.scalar.activation(
                    t1[:, do, :], ps, AF.Relu, bias=boc[:, do : do + 1]
                )
                nc.vector.tensor_tensor(
                    o2[:, do, :], t1[:, do, :], oln[:, do, :], ALU.add
                )
                nc.vector.tensor_tensor(
                    o2sq[:, do, :], o2[:, do, :], o2[:, do, :], ALU.mult
                )
                if do >= 1:
                    stat_mms(do - 1)
            stat_mms(7)

            # ---- LN1 (chunk-wise output DMA overlaps the apply) ----
            mb16b, rb16b = ln_stats(o2, (l1s1, l1s2), "ln1")
            xhat1 = ln_xhat(o2, mb16b, rb16b, "ln1")
            o3 = wp.tile([128, 8, NC_], f32)
            for dc in range(8):
                nc.scalar.activation(
                    o3[:, dc, :],
                    xhat1[:, dc, :],
                    AF.Identity,
                    scale=g1c[:, dc : dc + 1],
                    bias=be1c[:, dc : dc + 1],
                )
                nc.sync.dma_start(OUT2[:, dc, :], o3[:, dc, :])

    nc.compile()
    return nc


def _run(nc, in_maps, trace=False):
    from concourse.bass_utils import run_bass_kernel_spmd

    return run_bass_kernel_spmd(nc, in_maps, list(range(NCORES)), trace=trace)


def kernel(**inputs):
    trace = bool(int(__import__("os").environ.get("KERNEL_TRACE", "0")))
    f32 = np.float32
    bf16 = ml_dtypes.bfloat16
    f8 = ml_dtypes.float8_e4m3fn
    Q = np.ascontiguousarray(inputs["Q"], dtype=f32)
    K = np.ascontiguousarray(inputs["K"], dtype=f32)
    Wq, Wk, Wv, Wo = (np.asarray(inputs[k], f32) for k in ("Wq", "Wk", "Wv", "Wo"))
    bq, bk, bv, bo = (np.asarray(inputs[k], f32) for k in ("bq", "bk", "bv", "bo"))
    g0, be0, g1, be1 = (np.asarray(inputs[k], f32) for k in ("g0", "be0", "g1", "be1"))

    QT = np.ascontiguousarray(Q.transpose(0, 2, 1)).astype(bf16)
    KT8 = (
        np.ascontiguousarray(K.transpose(0, 2, 1)).reshape(B, 8, 128, N).astype(f8)
    )
    ones8 = np.full((128, 2, 128), OMEGA, dtype=f8)
    identb = np.eye(128, dtype=bf16)

    in_maps = []
    for h in range(H):
        hs = slice(h * DH, (h + 1) * DH)
        wqh = np.ascontiguousarray(
            Wq[:, hs].reshape(8, 128, 128).transpose(1, 0, 2)
        ).astype(bf16)
        wkh = np.ascontiguousarray(
            (Wk[:, hs] * 32.0).reshape(8, 128, 128).transpose(1, 0, 2)
        ).astype(f8)
        wvh = np.ascontiguousarray(
            (Wv[:, hs] * 32.0).reshape(8, 128, 128).transpose(1, 0, 2)
        ).astype(f8)
        in_maps.append(
            {
                "QT": QT,
                "KT8": KT8,
                "WQ": wqh,
                "WK": wkh,
                "WV": wvh,
                "BQ": bq[hs].reshape(128, 1).astype(f32),
                "BK": (bk[hs] * SCALE).reshape(128, 1).astype(f32),
                "BV": bv[hs].reshape(128, 1).astype(f32),
                "ONES8": ones8,
                "IDENTB": identb,
            }
        )

    r1 = _run(_build_d1(), in_maps, trace=trace)
    LAST_EXEC_NS["d1"] = r1.exec_time_ns

    # ---- host reshard: per-head [B, DH, N] f32 -> per-core [8, 128, 512] ----
    # core c <-> (batch c//2, n-half c%2); chunk dim = head index
    OTall = np.stack([r1.results[h]["OT"] for h in range(H)])  # [8, B, 128, N]
    wot_in = np.ascontiguousarray(
        Wo.reshape(8, 128, D).transpose(1, 0, 2)
    ).astype(bf16)
    onesb = np.ones((128, 128), dtype=bf16)
    col = lambda z: np.ascontiguousarray(z.reshape(8, 128).T, dtype=f32)
    boc, g0c, be0c, g1c, be1c = col(bo), col(g0), col(be0), col(g1), col(be1)
    in_maps2 = []
    for c in range(NCORES):
        b, nh = c // 2, c % 2
        # partition-major [128 p, 8 dc, 512 n] so the device DMA is contiguous
        xin = np.ascontiguousarray(
            OTall[:, b, :, nh * 512 : (nh + 1) * 512].transpose(1, 0, 2)
        ).astype(bf16)
        in_maps2.append(
            {
                "XIN": xin,
                "WOT": wot_in,
                "ONESB": onesb,
                "BO": boc,
                "G0": g0c,
                "BE0": be0c,
                "G1": g1c,
                "BE1": be1c,
                "EPSC": np.full((128, 1), LN_EPS, dtype=f32),
            }
        )
    r2 = _run(_build_d2(), in_maps2, trace=trace)
    LAST_EXEC_NS["d2"] = r2.exec_time_ns

    # ---- host unshard: [128 p, 8 dc, 512 n] -> [n, d] rows of O ----
    out = np.empty((B, N, D), dtype=f32)
    for c in range(NCORES):
        b, nh = c // 2, c % 2
        slab = r2.results[c]["OUT2"]  # [128, 8, 512] f32
        out[b, nh * 512 : (nh + 1) * 512, :] = (
            slab.transpose(2, 1, 0).reshape(512, D)
        )
    return out


# revision 73
# speedup vs baseline: 1.1396x; 1.0699x over previous
"""Trainium2 Bass kernel for nn_MABSINK (multi-head attention w/ 1-step Sinkhorn,
residuals, LayerNorms, fused output MLP).

Sharding: tensor-parallel over heads (8 heads -> 8 cores) for projections +
attention (dispatch 1); column-parallel over (batch, n-half) for LN0 + fc_o +
LN1 in the TRANSPOSED orientation (dispatch 2) so no on-chip transposes are
needed anywhere in dispatch 2 (d1's outputs are already [d, n]).

Both dispatches are software-pipelined: per-engine program order is interleaved
across batch/column tiles so the PE never sits behind a vector-chain of the
previous tile.

d1 math (eps=1, mu=nu):
  E = exp(S^T) stored [m, n] fp8;  omega*R[n] broadcast to all PSUM partitions
  via an all-ones fp8 DoubleRow matmul;  abc = 1/(omega*R) (fast reciprocal)
  ETa = E*abc in-place (accum_out -> cw);  vv = v/(omega*cw)
  attn^T via fp8 DoubleRow matmul;  o^T = MU*omega*attn + q^T (f32 residual)
d2: LN stats via broadcast all-ones bf16 matmuls (PE), apply on DVE/Scalar,
fc_o with transposed Wo chunks, fused bias+relu in PSUM evacuation.
"""

import functools
import math

import ml_dtypes
import numpy as np

B, N, D, H, DH = 4, 1024, 1024, 8, 128
MU = 1.0 / N + 1e-8  # == nu
LN_EPS = 1e-5
SCALE = 1.0 / math.sqrt(D)  # 1/32
NCORES = 8
OMEGA = 1.0 / 128.0  # fp8-range shift: ETa = E/(omega*R) lands in [0.03, 0.7]

LAST_EXEC_NS = {"d1": None, "d2": None}


def _mk_nc():
    import concourse.bacc as bacc

    return bacc.Bacc(
        "TRN2",
        target_bir_lowering=False,
        debug=False,
        enable_asserts=False,
        num_devices=NCORES,
    )


@functools.cache
def _build_d1():
    """Dispatch 1: projections + sinkhorn attention for one head (= one core).

    out[b] = (q + attn)^T as [DH, N] f32 per batch."""
    import concourse.bass as bass  # noqa: F401
    import concourse.mybir as mybir
    import concourse.tile as tile

    f32 = mybir.dt.float32
    bf16 = mybir.dt.bfloat16
    f8 = mybir.dt.float8e4
    AF = mybir.ActivationFunctionType
    ALU = mybir.AluOpType
    DR = mybir.MatmulPerfMode.DoubleRow

    nc = _mk_nc()
    QT = nc.dram_tensor("QT", [B, D, N], bf16, kind="ExternalInput").ap()
    KT8 = nc.dram_tensor("KT8", [B, 8, 128, N], f8, kind="ExternalInput").ap()
    WQ = nc.dram_tensor("WQ", [128, 8, 128], bf16, kind="ExternalInput").ap()
    WK = nc.dram_tensor("WK", [128, 8, 128], f8, kind="ExternalInput").ap()
    WV = nc.dram_tensor("WV", [128, 8, 128], f8, kind="ExternalInput").ap()
    BQ = nc.dram_tensor("BQ", [128, 1], f32, kind="ExternalInput").ap()
    BK = nc.dram_tensor("BK", [128, 1], f32, kind="ExternalInput").ap()
    BV = nc.dram_tensor("BV", [128, 1], f32, kind="ExternalInput").ap()
    ONES8 = nc.dram_tensor("ONES8", [128, 2, 128], f8, kind="ExternalInput").ap()
    IDENTB = nc.dram_tensor("IDENTB", [128, 128], bf16, kind="ExternalInput").ap()
    OT = nc.dram_tensor("OT", [B, DH, N], f32, kind="ExternalOutput").ap()

    with tile.TileContext(nc) as tc:
        with (
            tc.tile_pool(name="const", bufs=1) as constp,
            tc.tile_pool(name="io", bufs=6) as iop,
            tc.tile_pool(name="kt", bufs=2) as ktp,
            tc.tile_pool(name="mid", bufs=2) as midp,
            tc.tile_pool(name="et", bufs=2) as etp,
            tc.tile_pool(name="ps_proj", bufs=2, space="PSUM") as ps_proj,
            tc.tile_pool(name="ps_st", bufs=2, space="PSUM") as ps_st,
            tc.tile_pool(name="ps_r", bufs=2, space="PSUM") as ps_r,
            tc.tile_pool(name="ps_ot", bufs=2, space="PSUM") as ps_ot,
        ):
            # first batch's q inputs go on the DMA queue before everything else
            # (q-proj starts immediately; k-proj only needs ktc at ~+10us)
            qtcs0 = []
            for kc in range(8):
                t = iop.tile([128, N], bf16, tag="qtc", name=f"qtc0_{kc}")
                qtcs0.append(t)
            nc.sync.dma_start(qtcs0[0], QT[0, 0:128, :])

            wq = constp.tile([128, 8, 128], bf16)
            wk = constp.tile([128, 8, 128], f8)
            wv = constp.tile([128, 8, 128], f8)
            bq = constp.tile([128, 1], f32)
            bk = constp.tile([128, 1], f32)
            bv = constp.tile([128, 1], f32)
            ones8 = constp.tile([128, 2, 128], f8)
            identb = constp.tile([128, 128], bf16)
            nc.sync.dma_start(wq, WQ)
            nc.sync.dma_start(bq, BQ)
            for kc in range(1, 8):
                nc.sync.dma_start(qtcs0[kc], QT[0, kc * 128 : (kc + 1) * 128, :])
            # first KT in chunk-pairs so k-proj's first DR matmul starts as
            # soon as pair 0 lands instead of after the full 1MB
            ktc0 = ktp.tile([128, 8, N], f8, tag="ktc", name="ktc0")
            for t in range(4):
                nc.sync.dma_start(
                    ktc0[:, 2 * t : 2 * t + 2, :],
                    KT8[0, 2 * t : 2 * t + 2].rearrange("kc p n -> p kc n"),
                )
            nc.sync.dma_start(wk, WK)
            nc.sync.dma_start(wv, WV)
            nc.sync.dma_start(bk, BK)
            nc.sync.dma_start(bv, BV)
            nc.sync.dma_start(ones8, ONES8)
            nc.sync.dma_start(identb, IDENTB)
            # dummy exp to preload the activation table while DMAs run
            scratch1 = constp.tile([128, 1], f32)
            nc.scalar.activation(scratch1, bq, AF.Exp)

            def phase_a(b):
                """PE-heavy: projections, S+exp, v-transpose, omega*R."""
                if b == 0:
                    ktc = ktc0
                else:
                    ktc = ktp.tile([128, 8, N], f8, tag="ktc", name=f"ktc{b}")
                    nc.sync.dma_start(ktc, KT8[b].rearrange("kc p n -> p kc n"))

                qps = [
                    ps_proj.tile([128, 512], f32, tag="proj", name=f"qps{b}_{i}")
                    for i in range(2)
                ]
                for kc in range(8):
                    if b == 0:
                        qtc = qtcs0[kc]
                    else:
                        qtc = iop.tile(
                            [128, N], bf16, tag="qtc", name=f"qtc{b}_{kc}"
                        )
                        nc.sync.dma_start(qtc, QT[b, kc * 128 : (kc + 1) * 128, :])
                    for hf in range(2):
                        nc.tensor.matmul(
                            qps[hf],
                            wq[:, kc, :],
                            qtc[:, hf * 512 : (hf + 1) * 512],
                            start=(kc == 0),
                            stop=(kc == 7),
                        )
                # single bf16 evacuation serves both S and the residual (the
                # bf16 q rounding is washed out by d2's bf16 input anyway)
                qT = midp.tile([128, N], bf16, tag="qT", name=f"qT{b}")
                for hf in range(2):
                    hs = slice(hf * 512, (hf + 1) * 512)
                    nc.scalar.activation(qT[:, hs], qps[hf], AF.Identity, bias=bq)

                kps = [
                    ps_proj.tile([128, 512], f32, tag="proj", name=f"kps{b}_{i}")
                    for i in range(2)
                ]
                for t in range(4):
                    for hf in range(2):
                        nc.tensor.matmul(
                            kps[hf],
                            wk[:, 2 * t : 2 * t + 2, :],
                            ktc[:, 2 * t : 2 * t + 2, hf * 512 : (hf + 1) * 512],
                            start=(t == 0),
                            stop=(t == 3),
                            perf_mode=DR,
                        )
                kT = midp.tile([128, N], bf16, tag="kT", name=f"kT{b}")
                for hf in range(2):
                    nc.scalar.activation(
                        kT[:, hf * 512 : (hf + 1) * 512],
                        kps[hf],
                        AF.Identity,
                        bias=bk,
                        scale=1.0 / 1024.0,
                    )

                vps = [
                    ps_proj.tile([128, 512], f32, tag="proj", name=f"vps{b}_{i}")
                    for i in range(2)
                ]
                for t in range(4):
                    for hf in range(2):
                        nc.tensor.matmul(
                            vps[hf],
                            wv[:, 2 * t : 2 * t + 2, :],
                            ktc[:, 2 * t : 2 * t + 2, hf * 512 : (hf + 1) * 512],
                            start=(t == 0),
                            stop=(t == 3),
                            perf_mode=DR,
                        )
                vT = midp.tile([128, N], bf16, tag="vT", name=f"vT{b}")
                for hf in range(2):
                    nc.scalar.activation(
                        vT[:, hf * 512 : (hf + 1) * 512],
                        vps[hf],
                        AF.Identity,
                        bias=bv,
                        scale=1.0 / 32.0,
                    )
                ET = etp.tile([128, 8, N], f8, tag="ET", name=f"ET{b}")
                for mt in range(8):
                    for hf in range(2):
                        sps = ps_st.tile(
                            [128, 512], f32, tag="st", name=f"sps{b}_{mt}_{hf}"
                        )
                        nc.tensor.matmul(
                            sps,
                            kT[:, mt * 128 : (mt + 1) * 128],
                            qT[:, hf * 512 : (hf + 1) * 512],
                            start=True,
                            stop=True,
                        )
                        nc.scalar.activation(
                            ET[:, mt, hf * 512 : (hf + 1) * 512], sps, AF.Exp
                        )

                # v = vT^T via PE transposes (hidden under the exp shadow)
                v = midp.tile([128, 8, 128], bf16, tag="v", name=f"v{b}")
                for mt in range(8):
                    tp = ps_st.tile([128, 128], bf16, tag="st", name=f"tp{b}_{mt}")
                    nc.tensor.transpose(
                        tp, vT[:, mt * 128 : (mt + 1) * 128], identb
                    )
                    nc.vector.tensor_copy(v[:, mt, :], tp)

                # omega*R broadcast to every PSUM partition (all-ones DR mm);
                # the reciprocal is deferred to phase_b_dve
                rpss = []
                for hf in range(2):
                    rps = ps_r.tile([128, 512], f32, tag="r", name=f"rps{b}_{hf}")
                    for t in range(4):
                        nc.tensor.matmul(
                            rps,
                            ones8,
                            ET[:, 2 * t : 2 * t + 2, hf * 512 : (hf + 1) * 512],
                            start=(t == 0),
                            stop=(t == 3),
                            perf_mode=DR,
                        )
                    rpss.append(rps)
                return dict(qT=qT, rpss=rpss, ET=ET, v=v)

            def phase_b(s, b):
                """Per-mt interleaved: ETa (+c accum) -> vv -> attn pair, then
                residual + output DMA."""
                ET, v, qT = s["ET"], s["v"], s["qT"]
                abcf = midp.tile([128, N], f32, tag="abcf", name=f"abcf{b}")
                for hf in range(2):
                    nc.vector.reciprocal_approx_fast(
                        abcf[:, hf * 512 : (hf + 1) * 512], s["rpss"][hf]
                    )
                # bf16 copy: 16-bit in1 makes the fp8 STT pass cheaper on DVE
                abc = midp.tile([128, N], bf16, tag="abc", name=f"abc{b}")
                nc.scalar.activation(abc, abcf, AF.Copy)
                cw = midp.tile([128, 8], f32, tag="cw", name=f"cw{b}")
                wrec = midp.tile([128, 8], f32, tag="wrec", name=f"wrec{b}")
                vv = midp.tile([128, 8, 128], f8, tag="vv", name=f"vv{b}")
                ops_ = [
                    ps_ot.tile([128, 512], f32, tag="ot", name=f"ot{b}_{i}")
                    for i in range(2)
                ]
                for t in range(4):
                    for j in range(2):
                        mt = 2 * t + j
                        nc.vector.scalar_tensor_tensor(
                            out=ET[:, mt, :],
                            in0=ET[:, mt, :],
                            scalar=1.0,
                            in1=abc,
                            op0=ALU.mult,
                            op1=ALU.mult,
                            accum_out=cw[:, mt : mt + 1],
                        )
                        nc.vector.reciprocal_approx_fast(
                            wrec[:, mt : mt + 1], cw[:, mt : mt + 1]
                        )
                        nc.vector.tensor_scalar(
                            out=vv[:, mt, :],
                            in0=v[:, mt, :],
                            scalar1=wrec[:, mt : mt + 1],
                            scalar2=1.0 / OMEGA,
                            op0=ALU.mult,
                            op1=ALU.mult,
                        )
                    for hf in range(2):
                        nc.tensor.matmul(
                            ops_[hf],
                            vv[:, 2 * t : 2 * t + 2, :],
                            ET[:, 2 * t : 2 * t + 2, hf * 512 : (hf + 1) * 512],
                            start=(t == 0),
                            stop=(t == 3),
                            perf_mode=DR,
                        )
                ofin = midp.tile([128, N], f32, tag="ofin", name=f"ofin{b}")
                for hf in range(2):
                    hs = slice(hf * 512, (hf + 1) * 512)
                    nc.vector.scalar_tensor_tensor(
                        out=ofin[:, hs],
                        in0=ops_[hf],
                        scalar=float(MU) * OMEGA,
                        in1=qT[:, hs],
                        op0=ALU.mult,
                        op1=ALU.add,
                    )
                nc.sync.dma_start(OT[b], ofin)

            # pipeline: A0 A1 B0 A2 B1 A3 B2 B3
            state = []
            for b in range(B):
                state.append(phase_a(b))
                if b >= 1:
                    phase_b(state[b - 1], b - 1)
            phase_b(state[B - 1], B - 1)

    nc.compile()
    return nc


@functools.cache
def _build_d2():
    """Dispatch 2 (transposed): LN0 -> fc_o(+relu, residual) -> LN1 on a
    [1024 d, 512 n] column slab per core. LN stats via broadcast all-ones
    matmuls; no on-chip transposes."""
    import concourse.mybir as mybir
    import concourse.tile as tile

    f32 = mybir.dt.float32
    bf16 = mybir.dt.bfloat16
    AF = mybir.ActivationFunctionType
    ALU = mybir.AluOpType

    NC_ = 512  # full column slab; fc_o inner stages are pipelined per do-chunk

    nc = _mk_nc()
    XIN = nc.dram_tensor("XIN", [128, 8, 512], bf16, kind="ExternalInput").ap()
    WOT = nc.dram_tensor("WOT", [128, 8, D], bf16, kind="ExternalInput").ap()
    ONESB = nc.dram_tensor("ONESB", [128, 128], bf16, kind="ExternalInput").ap()
    BO = nc.dram_tensor("BO", [128, 8], f32, kind="ExternalInput").ap()
    G0 = nc.dram_tensor("G0", [128, 8], f32, kind="ExternalInput").ap()
    BE0 = nc.dram_tensor("BE0", [128, 8], f32, kind="ExternalInput").ap()
    G1 = nc.dram_tensor("G1", [128, 8], f32, kind="ExternalInput").ap()
    BE1 = nc.dram_tensor("BE1", [128, 8], f32, kind="ExternalInput").ap()
    EPSC = nc.dram_tensor("EPSC", [128, 1], f32, kind="ExternalInput").ap()
    OUT2 = nc.dram_tensor("OUT2", [128, 8, 512], f32, kind="ExternalOutput").ap()

    with tile.TileContext(nc) as tc:
        with (
            tc.tile_pool(name="const", bufs=1) as constp,
            tc.tile_pool(name="work", bufs=1) as wp,
            tc.tile_pool(name="small", bufs=2) as sp,
            tc.tile_pool(name="ps_mm", bufs=2, space="PSUM") as ps_mm,
            tc.tile_pool(name="ps_s", bufs=2, space="PSUM") as ps_s,
            tc.tile_pool(name="ps_l1", bufs=2, space="PSUM") as ps_l1,
        ):
            wot = constp.tile([128, 8, D], bf16)
            onesb = constp.tile([128, 128], bf16)
            boc = constp.tile([128, 8], f32)
            g0c = constp.tile([128, 8], f32)
            be0c = constp.tile([128, 8], f32)
            g1c = constp.tile([128, 8], f32)
            be1c = constp.tile([128, 8], f32)
            epsc = constp.tile([128, 1], f32)
            x = wp.tile([128, 8, NC_], bf16)
            nc.sync.dma_start(x, XIN)
            nc.sync.dma_start(onesb, ONESB)
            nc.sync.dma_start(epsc, EPSC)
            nc.sync.dma_start(wot, WOT)
            nc.sync.dma_start(boc, BO)
            nc.sync.dma_start(g0c, G0)
            nc.sync.dma_start(be0c, BE0)
            nc.sync.dma_start(g1c, G1)
            nc.sync.dma_start(be1c, BE1)

            def ln_stats(x_in, sq_in, uid):
                """Broadcast mean/rstd [128, NC_] from psum-accumulated
                column sums. Short chain: var folded into the Sqrt activation
                (scale=1/D, bias=eps), mean cast fused with its 1/D scale."""
                ps1, ps2 = sq_in
                mb16 = sp.tile([128, 1, NC_], bf16, tag="mb16", name=f"mb16_{uid}")
                nc.scalar.activation(
                    mb16[:, 0, :], ps1, AF.Copy, scale=1.0 / D
                )
                v1 = sp.tile([128, NC_], f32, tag="v1", name=f"v1_{uid}")
                nc.vector.scalar_tensor_tensor(
                    out=v1,
                    in0=ps1,
                    scalar=1.0,
                    in1=mb16[:, 0, :],
                    op0=ALU.mult,
                    op1=ALU.mult,
                )
                dif = sp.tile([128, NC_], f32, tag="dif", name=f"dif_{uid}")
                nc.vector.tensor_tensor(dif, ps2, v1, ALU.subtract)
                sq = sp.tile([128, NC_], f32, tag="sq", name=f"sq_{uid}")
                nc.scalar.activation(
                    sq, dif, AF.Sqrt, scale=1.0 / D, bias=epsc
                )
                rstd = sp.tile([128, NC_], f32, tag="rstd", name=f"rstd_{uid}")
                nc.vector.reciprocal_approx_fast(rstd, sq)
                rb16 = sp.tile([128, 1, NC_], bf16, tag="rb16", name=f"rb16_{uid}")
                nc.scalar.activation(rb16[:, 0, :], rstd, AF.Copy)
                return mb16, rb16

            def ln_xhat(x_in, mb16, rb16, uid):
                """(x - mean) * rstd, per chunk (stride-0 broadcast DVE reads
                measured slower than per-chunk ops)."""
                xr = sp.tile([128, 8, NC_], bf16, tag="xra", name=f"xra_{uid}")
                for dc in range(8):
                    xc = sp.tile(
                        [128, NC_], bf16, tag="xca", name=f"xca_{uid}_{dc}"
                    )
                    nc.vector.tensor_tensor(
                        xc, x_in[:, dc, :], mb16[:, 0, :], ALU.subtract
                    )
                    nc.vector.tensor_tensor(
                        xr[:, dc, :], xc, rb16[:, 0, :], ALU.mult
                    )
                return xr

            # ---- LN0: stats then apply ----
            ps1 = ps_s.tile([128, NC_], f32, tag="s", name="ps1_ln0")
            for dc in range(8):
                nc.tensor.matmul(
                    ps1, onesb, x[:, dc, :], start=(dc == 0), stop=(dc == 7)
                )
            xsq = sp.tile([128, 8, NC_], bf16, tag="xsq", name="xsq_ln0")
            nc.vector.tensor_tensor(xsq, x, x, ALU.mult)
            ps2 = ps_s.tile([128, NC_], f32, tag="s", name="ps2_ln0")
            for dc in range(8):
                nc.tensor.matmul(
                    ps2, onesb, xsq[:, dc, :], start=(dc == 0), stop=(dc == 7)
                )
            mb16, rb16 = ln_stats(x, (ps1, ps2), "ln0")
            xhat0 = ln_xhat(x, mb16, rb16, "ln0")
            oln = wp.tile([128, 8, NC_], bf16)
            for dc in range(8):
                nc.scalar.activation(
                    oln[:, dc, :],
                    xhat0[:, dc, :],
                    AF.Identity,
                    scale=g0c[:, dc : dc + 1],
                    bias=be0c[:, dc : dc + 1],
                )

            # ---- fc_o with fused relu/residual/LN1-stat accumulation ----
            t1 = wp.tile([128, 8, NC_], bf16)
            o2 = wp.tile([128, 8, NC_], bf16)
            o2sq = wp.tile([128, 8, NC_], bf16)
            l1s1 = ps_l1.tile([128, NC_], f32, tag="l1", name="l1s1")
            l1s2 = ps_l1.tile([128, NC_], f32, tag="l1", name="l1s2")
            def stat_mms(do):
                # LN1 stat contribution for chunk do (emitted one iteration
                # late so the PE never waits on the relu->residual chain)
                nc.tensor.matmul(
                    l1s1,
                    onesb,
                    o2[:, do, :],
                    start=(do == 0),
                    stop=(do == 7),
                    skip_group_check=True,
                )
                nc.tensor.matmul(
                    l1s2,
                    onesb,
                    o2sq[:, do, :],
                    start=(do == 0),
                    stop=(do == 7),
                    skip_group_check=True,
                )

            for do in range(8):
                ps = ps_mm.tile([128, NC_], f32, tag="mm", name=f"mm{do}")
                for di in range(8):
                    nc.tensor.matmul(
                        ps,
                        wot[:, di, do * 128 : (do + 1) * 128],
                        oln[:, di, :],
                        start=(di == 0),
                        stop=(di == 7),
                    )
                nc.scalar.activation(
                    t1[:, do, :], ps, AF.Relu, bias=boc[:, do : do + 1]
                )
                nc.vector.tensor_tensor(
                    o2[:, do, :], t1[:, do, :], oln[:, do, :], ALU.add
                )
                nc.vector.tensor_tensor(
                    o2sq[:, do, :], o2[:, do, :], o2[:, do, :], ALU.mult
                )
                if do >= 1:
                    stat_mms(do - 1)
            stat_mms(7)

            # ---- LN1 (chunk-wise output DMA overlaps the apply) ----
            mb16b, rb16b = ln_stats(o2, (l1s1, l1s2), "ln1")
            xhat1 = ln_xhat(o2, mb16b, rb16b, "ln1")
            o3 = wp.tile([128, 8, NC_], f32)
            for dc in range(8):
                nc.scalar.activation(
                    o3[:, dc, :],
                    xhat1[:, dc, :],
                    AF.Identity,
                    scale=g1c[:, dc : dc + 1],
                    bias=be1c[:, dc : dc + 1],
                )
                nc.sync.dma_start(OUT2[:, dc, :], o3[:, dc, :])

    nc.compile()
    return nc


def _run(nc, in_maps, trace=False):
    from concourse.bass_utils import run_bass_kernel_spmd

    return run_bass_kernel_spmd(nc, in_maps, list(range(NCORES)), trace=trace)


def kernel(**inputs):
    trace = bool(int(__import__("os").environ.get("KERNEL_TRACE", "0")))
    f32 = np.float32
    bf16 = ml_dtypes.bfloat16
    f8 = ml_dtypes.float8_e4m3fn
    Q = np.ascontiguousarray(inputs["Q"], dtype=f32)
    K = np.ascontiguousarray(inputs["K"], dtype=f32)
    Wq, Wk, Wv, Wo = (np.asarray(inputs[k], f32) for k in ("Wq", "Wk", "Wv", "Wo"))
    bq, bk, bv, bo = (np.asarray(inputs[k], f32) for k in ("bq", "bk", "bv", "bo"))
    g0, be0, g1, be1 = (np.asarray(inputs[k], f32) for k in ("g0", "be0", "g1", "be1"))

    QT = np.ascontiguousarray(Q.transpose(0, 2, 1)).astype(bf16)
    KT8 = (
        np.ascontiguousarray(K.transpose(0, 2, 1)).reshape(B, 8, 128, N).astype(f8)
    )
    ones8 = np.full((128, 2, 128), OMEGA, dtype=f8)
    identb = np.eye(128, dtype=bf16)

    in_maps = []
    for h in range(H):
        hs = slice(h * DH, (h + 1) * DH)
        wqh = np.ascontiguousarray(
            Wq[:, hs].reshape(8, 128, 128).transpose(1, 0, 2)
        ).astype(bf16)
        wkh = np.ascontiguousarray(
            (Wk[:, hs] * 32.0).reshape(8, 128, 128).transpose(1, 0, 2)
        ).astype(f8)
        wvh = np.ascontiguousarray(
            (Wv[:, hs] * 32.0).reshape(8, 128, 128).transpose(1, 0, 2)
        ).astype(f8)
        in_maps.append(
            {
                "QT": QT,
                "KT8": KT8,
                "WQ": wqh,
                "WK": wkh,
                "WV": wvh,
                "BQ": bq[hs].reshape(128, 1).astype(f32),
                "BK": (bk[hs] * SCALE).reshape(128, 1).astype(f32),
                "BV": bv[hs].reshape(128, 1).astype(f32),
                "ONES8": ones8,
                "IDENTB": identb,
            }
        )

    r1 = _run(_build_d1(), in_maps, trace=trace)
    LAST_EXEC_NS["d1"] = r1.exec_time_ns

    # ---- host reshard: per-head [B, DH, N] f32 -> per-core [8, 128, 512] ----
    # core c <-> (batch c//2, n-half c%2); chunk dim = head index
    OTall = np.stack([r1.results[h]["OT"] for h in range(H)])  # [8, B, 128, N]
    wot_in = np.ascontiguousarray(
        Wo.reshape(8, 128, D).transpose(1, 0, 2)
    ).astype(bf16)
    onesb = np.ones((128, 128), dtype=bf16)
    col = lambda z: np.ascontiguousarray(z.reshape(8, 128).T, dtype=f32)
    boc, g0c, be0c, g1c, be1c = col(bo), col(g0), col(be0), col(g1), col(be1)
    in_maps2 = []
    for c in range(NCORES):
        b, nh = c // 2, c % 2
        # partition-major [128 p, 8 dc, 512 n] so the device DMA is contiguous
        xin = np.ascontiguousarray(
            OTall[:, b, :, nh * 512 : (nh + 1) * 512].transpose(1, 0, 2)
        ).astype(bf16)
        in_maps2.append(
            {
                "XIN": xin,
                "WOT": wot_in,
                "ONESB": onesb,
                "BO": boc,
                "G0": g0c,
                "BE0": be0c,
                "G1": g1c,
                "BE1": be1c,
                "EPSC": np.full((128, 1), LN_EPS, dtype=f32),
            }
        )
    r2 = _run(_build_d2(), in_maps2, trace=trace)
    LAST_EXEC_NS["d2"] = r2.exec_time_ns

    # ---- host unshard: [128 p, 8 dc, 512 n] -> [n, d] rows of O ----
    out = np.empty((B, N, D), dtype=f32)
    for c in range(NCORES):
        b, nh = c // 2, c % 2
        slab = r2.results[c]["OUT2"]  # [128, 8, 512] f32
        out[b, nh * 512 : (nh + 1) * 512, :] = (
            slab.transpose(2, 1, 0).reshape(512, D)
        )
    return out


# revision 74
# speedup vs baseline: 1.2272x; 1.0768x over previous
"""Trainium2 Bass kernel for nn_MABSINK (multi-head attention w/ 1-step Sinkhorn,
residuals, LayerNorms, fused output MLP).

Sharding: tensor-parallel over heads (8 heads -> 8 cores) for projections +
attention (dispatch 1); column-parallel over (batch, n-half) for LN0 + fc_o +
LN1 in the TRANSPOSED orientation (dispatch 2) so no on-chip transposes are
needed anywhere in dispatch 2 (d1's outputs are already [d, n]).

Both dispatches are software-pipelined: per-engine program order is interleaved
across batch/column tiles so the PE never sits behind a vector-chain of the
previous tile.

d1 math (eps=1, mu=nu):
  E = exp(S^T) stored [m, n] fp8;  omega*R[n] broadcast to all PSUM partitions
  via an all-ones fp8 DoubleRow matmul;  abc = 1/(omega*R) (fast reciprocal)
  ETa = E*abc in-place (accum_out -> cw);  vv = v/(omega*cw)
  attn^T via fp8 DoubleRow matmul;  o^T = MU*omega*attn + q^T (f32 residual)
d2: LN stats via broadcast all-ones bf16 matmuls (PE), apply on DVE/Scalar,
fc_o with transposed Wo chunks, fused bias+relu in PSUM evacuation.
"""

import functools
import math

import ml_dtypes
import numpy as np

B, N, D, H, DH = 4, 1024, 1024, 8, 128
MU = 1.0 / N + 1e-8  # == nu
LN_EPS = 1e-5
SCALE = 1.0 / math.sqrt(D)  # 1/32
NCORES = 8
OMEGA = 1.0 / 128.0  # fp8-range shift: ETa = E/(omega*R) lands in [0.03, 0.7]

LAST_EXEC_NS = {"d1": None, "d2": None}


def _mk_nc():
    import concourse.bacc as bacc

    return bacc.Bacc(
        "TRN2",
        target_bir_lowering=False,
        debug=False,
        enable_asserts=False,
        num_devices=NCORES,
    )


@functools.cache
def _build_d1():
    """Dispatch 1: projections + sinkhorn attention for one head (= one core).

    out[b] = (q + attn)^T as [DH, N] f32 per batch."""
    import concourse.bass as bass  # noqa: F401
    import concourse.mybir as mybir
    import concourse.tile as tile

    f32 = mybir.dt.float32
    bf16 = mybir.dt.bfloat16
    f8 = mybir.dt.float8e4
    AF = mybir.ActivationFunctionType
    ALU = mybir.AluOpType
    DR = mybir.MatmulPerfMode.DoubleRow

    nc = _mk_nc()
    QT = nc.dram_tensor("QT", [B, D, N], bf16, kind="ExternalInput").ap()
    KT8 = nc.dram_tensor("KT8", [B, 8, 128, N], f8, kind="ExternalInput").ap()
    WQ = nc.dram_tensor("WQ", [128, 8, 128], bf16, kind="ExternalInput").ap()
    WK = nc.dram_tensor("WK", [128, 8, 128], f8, kind="ExternalInput").ap()
    WV = nc.dram_tensor("WV", [128, 8, 128], f8, kind="ExternalInput").ap()
    BQ = nc.dram_tensor("BQ", [128, 1], f32, kind="ExternalInput").ap()
    BK = nc.dram_tensor("BK", [128, 1], f32, kind="ExternalInput").ap()
    BV = nc.dram_tensor("BV", [128, 1], f32, kind="ExternalInput").ap()
    ONES8 = nc.dram_tensor("ONES8", [128, 2, 128], f8, kind="ExternalInput").ap()
    IDENTB = nc.dram_tensor("IDENTB", [128, 128], bf16, kind="ExternalInput").ap()
    OT = nc.dram_tensor("OT", [B, DH, N], f32, kind="ExternalOutput").ap()

    with tile.TileContext(nc) as tc:
        with (
            tc.tile_pool(name="const", bufs=1) as constp,
            tc.tile_pool(name="io", bufs=6) as iop,
            tc.tile_pool(name="kt", bufs=2) as ktp,
            tc.tile_pool(name="mid", bufs=2) as midp,
            tc.tile_pool(name="et", bufs=2) as etp,
            tc.tile_pool(name="ps_proj", bufs=2, space="PSUM") as ps_proj,
            tc.tile_pool(name="ps_st", bufs=2, space="PSUM") as ps_st,
            tc.tile_pool(name="ps_r", bufs=2, space="PSUM") as ps_r,
            tc.tile_pool(name="ps_ot", bufs=2, space="PSUM") as ps_ot,
        ):
            # first batch's q inputs go on the DMA queue before everything else
            # (q-proj starts immediately; k-proj only needs ktc at ~+10us)
            qtcs0 = []
            for kc in range(8):
                t = iop.tile([128, N], bf16, tag="qtc", name=f"qtc0_{kc}")
                qtcs0.append(t)
            nc.sync.dma_start(qtcs0[0], QT[0, 0:128, :])

            wq = constp.tile([128, 8, 128], bf16)
            wk = constp.tile([128, 8, 128], f8)
            wv = constp.tile([128, 8, 128], f8)
            bq = constp.tile([128, 1], f32)
            bk = constp.tile([128, 1], f32)
            bv = constp.tile([128, 1], f32)
            ones8 = constp.tile([128, 2, 128], f8)
            identb = constp.tile([128, 128], bf16)
            nc.sync.dma_start(wq, WQ)
            nc.sync.dma_start(bq, BQ)
            for kc in range(1, 8):
                nc.sync.dma_start(qtcs0[kc], QT[0, kc * 128 : (kc + 1) * 128, :])
            # first KT in chunk-pairs so k-proj's first DR matmul starts as
            # soon as pair 0 lands instead of after the full 1MB
            ktc0 = ktp.tile([128, 8, N], f8, tag="ktc", name="ktc0")
            for t in range(4):
                nc.sync.dma_start(
                    ktc0[:, 2 * t : 2 * t + 2, :],
                    KT8[0, 2 * t : 2 * t + 2].rearrange("kc p n -> p kc n"),
                )
            nc.sync.dma_start(wk, WK)
            nc.sync.dma_start(wv, WV)
            nc.sync.dma_start(bk, BK)
            nc.sync.dma_start(bv, BV)
            nc.sync.dma_start(ones8, ONES8)
            nc.sync.dma_start(identb, IDENTB)
            # dummy exp to preload the activation table while DMAs run
            scratch1 = constp.tile([128, 1], f32)
            nc.scalar.activation(scratch1, bq, AF.Exp)

            def phase_a(b):
                """PE-heavy: projections, S+exp, v-transpose, omega*R."""
                if b == 0:
                    ktc = ktc0
                else:
                    ktc = ktp.tile([128, 8, N], f8, tag="ktc", name=f"ktc{b}")
                    nc.sync.dma_start(ktc, KT8[b].rearrange("kc p n -> p kc n"))

                qps = [
                    ps_proj.tile([128, 512], f32, tag="proj", name=f"qps{b}_{i}")
                    for i in range(2)
                ]
                for kc in range(8):
                    if b == 0:
                        qtc = qtcs0[kc]
                    else:
                        qtc = iop.tile(
                            [128, N], bf16, tag="qtc", name=f"qtc{b}_{kc}"
                        )
                        nc.sync.dma_start(qtc, QT[b, kc * 128 : (kc + 1) * 128, :])
                    for hf in range(2):
                        nc.tensor.matmul(
                            qps[hf],
                            wq[:, kc, :],
                            qtc[:, hf * 512 : (hf + 1) * 512],
                            start=(kc == 0),
                            stop=(kc == 7),
                        )
                # single bf16 evacuation serves both S and the residual (the
                # bf16 q rounding is washed out by d2's bf16 input anyway)
                qT = midp.tile([128, N], bf16, tag="qT", name=f"qT{b}")
                for hf in range(2):
                    hs = slice(hf * 512, (hf + 1) * 512)
                    nc.scalar.activation(qT[:, hs], qps[hf], AF.Identity, bias=bq)

                kps = [
                    ps_proj.tile([128, 512], f32, tag="proj", name=f"kps{b}_{i}")
                    for i in range(2)
                ]
                for t in range(4):
                    for hf in range(2):
                        nc.tensor.matmul(
                            kps[hf],
                            wk[:, 2 * t : 2 * t + 2, :],
                            ktc[:, 2 * t : 2 * t + 2, hf * 512 : (hf + 1) * 512],
                            start=(t == 0),
                            stop=(t == 3),
                            perf_mode=DR,
                        )
                kT = midp.tile([128, N], bf16, tag="kT", name=f"kT{b}")
                for hf in range(2):
                    nc.scalar.activation(
                        kT[:, hf * 512 : (hf + 1) * 512],
                        kps[hf],
                        AF.Identity,
                        bias=bk,
                        scale=1.0 / 1024.0,
                    )

                vps = [
                    ps_proj.tile([128, 512], f32, tag="proj", name=f"vps{b}_{i}")
                    for i in range(2)
                ]
                for t in range(4):
                    for hf in range(2):
                        nc.tensor.matmul(
                            vps[hf],
                            wv[:, 2 * t : 2 * t + 2, :],
                            ktc[:, 2 * t : 2 * t + 2, hf * 512 : (hf + 1) * 512],
                            start=(t == 0),
                            stop=(t == 3),
                            perf_mode=DR,
                        )
                vT = midp.tile([128, N], bf16, tag="vT", name=f"vT{b}")
                for hf in range(2):
                    nc.scalar.activation(
                        vT[:, hf * 512 : (hf + 1) * 512],
                        vps[hf],
                        AF.Identity,
                        bias=bv,
                        scale=1.0 / 32.0,
                    )
                ET = etp.tile([128, 8, N], f8, tag="ET", name=f"ET{b}")
                for mt in range(8):
                    for hf in range(2):
                        sps = ps_st.tile(
                            [128, 512], f32, tag="st", name=f"sps{b}_{mt}_{hf}"
                        )
                        nc.tensor.matmul(
                            sps,
                            kT[:, mt * 128 : (mt + 1) * 128],
                            qT[:, hf * 512 : (hf + 1) * 512],
                            start=True,
                            stop=True,
                        )
                        nc.scalar.activation(
                            ET[:, mt, hf * 512 : (hf + 1) * 512], sps, AF.Exp
                        )

                # v = vT^T via PE transposes (hidden under the exp shadow)
                v = midp.tile([128, 8, 128], bf16, tag="v", name=f"v{b}")
                for mt in range(8):
                    tp = ps_st.tile([128, 128], bf16, tag="st", name=f"tp{b}_{mt}")
                    nc.tensor.transpose(
                        tp, vT[:, mt * 128 : (mt + 1) * 128], identb
                    )
                    nc.vector.tensor_copy(v[:, mt, :], tp)

                # omega*R broadcast to every PSUM partition (all-ones DR mm);
                # the reciprocal is deferred to phase_b_dve
                rpss = []
                for hf in range(2):
                    rps = ps_r.tile([128, 512], f32, tag="r", name=f"rps{b}_{hf}")
                    for t in range(4):
                        nc.tensor.matmul(
                            rps,
                            ones8,
                            ET[:, 2 * t : 2 * t + 2, hf * 512 : (hf + 1) * 512],
                            start=(t == 0),
                            stop=(t == 3),
                            perf_mode=DR,
                        )
                    rpss.append(rps)
                return dict(qT=qT, rpss=rpss, ET=ET, v=v)

            def phase_b(s, b):
                """Per-mt interleaved: ETa (+c accum) -> vv -> attn pair, then
                residual + output DMA."""
                ET, v, qT = s["ET"], s["v"], s["qT"]
                abcf = midp.tile([128, N], f32, tag="abcf", name=f"abcf{b}")
                for hf in range(2):
                    nc.vector.reciprocal_approx_fast(
                        abcf[:, hf * 512 : (hf + 1) * 512], s["rpss"][hf]
                    )
                # bf16 copy: 16-bit in1 makes the fp8 STT pass cheaper on DVE
                # no bf16 cast: a scalar-queue cast would sit behind the
                # next batch's exp activations and stall this B-chain
                abc = abcf
                cw = midp.tile([128, 8], f32, tag="cw", name=f"cw{b}")
                wrec = midp.tile([128, 8], f32, tag="wrec", name=f"wrec{b}")
                vv = midp.tile([128, 8, 128], f8, tag="vv", name=f"vv{b}")
                ops_ = [
                    ps_ot.tile([128, 512], f32, tag="ot", name=f"ot{b}_{i}")
                    for i in range(2)
                ]
                for t in range(4):
                    for j in range(2):
                        mt = 2 * t + j
                        nc.vector.scalar_tensor_tensor(
                            out=ET[:, mt, :],
                            in0=ET[:, mt, :],
                            scalar=1.0,
                            in1=abc,
                            op0=ALU.mult,
                            op1=ALU.mult,
                            accum_out=cw[:, mt : mt + 1],
                        )
                        nc.vector.reciprocal_approx_fast(
                            wrec[:, mt : mt + 1], cw[:, mt : mt + 1]
                        )
                        nc.vector.tensor_scalar(
                            out=vv[:, mt, :],
                            in0=v[:, mt, :],
                            scalar1=wrec[:, mt : mt + 1],
                            scalar2=1.0 / OMEGA,
                            op0=ALU.mult,
                            op1=ALU.mult,
                        )
                    for hf in range(2):
                        nc.tensor.matmul(
                            ops_[hf],
                            vv[:, 2 * t : 2 * t + 2, :],
                            ET[:, 2 * t : 2 * t + 2, hf * 512 : (hf + 1) * 512],
                            start=(t == 0),
                            stop=(t == 3),
                            perf_mode=DR,
                        )
                ofin = midp.tile([128, N], f32, tag="ofin", name=f"ofin{b}")
                for hf in range(2):
                    hs = slice(hf * 512, (hf + 1) * 512)
                    nc.vector.scalar_tensor_tensor(
                        out=ofin[:, hs],
                        in0=ops_[hf],
                        scalar=float(MU) * OMEGA,
                        in1=qT[:, hs],
                        op0=ALU.mult,
                        op1=ALU.add,
                    )
                nc.sync.dma_start(OT[b], ofin)

            # pipeline: A0 A1 B0 A2 B1 A3 B2 B3
            state = []
            for b in range(B):
                state.append(phase_a(b))
                if b >= 1:
                    phase_b(state[b - 1], b - 1)
            phase_b(state[B - 1], B - 1)

    nc.compile()
    return nc


@functools.cache
def _build_d2():
    """Dispatch 2 (transposed): LN0 -> fc_o(+relu, residual) -> LN1 on a
    [1024 d, 512 n] column slab per core. LN stats via broadcast all-ones
    matmuls; no on-chip transposes."""
    import concourse.mybir as mybir
    import concourse.tile as tile

    f32 = mybir.dt.float32
    bf16 = mybir.dt.bfloat16
    AF = mybir.ActivationFunctionType
    ALU = mybir.AluOpType

    NC_ = 512  # full column slab; fc_o inner stages are pipelined per do-chunk

    nc = _mk_nc()
    XIN = nc.dram_tensor("XIN", [128, 8, 512], bf16, kind="ExternalInput").ap()
    WOT = nc.dram_tensor("WOT", [128, 8, D], bf16, kind="ExternalInput").ap()
    ONESB = nc.dram_tensor("ONESB", [128, 128], bf16, kind="ExternalInput").ap()
    BO = nc.dram_tensor("BO", [128, 8], f32, kind="ExternalInput").ap()
    G0 = nc.dram_tensor("G0", [128, 8], f32, kind="ExternalInput").ap()
    BE0 = nc.dram_tensor("BE0", [128, 8], f32, kind="ExternalInput").ap()
    G1 = nc.dram_tensor("G1", [128, 8], f32, kind="ExternalInput").ap()
    BE1 = nc.dram_tensor("BE1", [128, 8], f32, kind="ExternalInput").ap()
    EPSC = nc.dram_tensor("EPSC", [128, 1], f32, kind="ExternalInput").ap()
    OUT2 = nc.dram_tensor("OUT2", [128, 8, 512], f32, kind="ExternalOutput").ap()

    with tile.TileContext(nc) as tc:
        with (
            tc.tile_pool(name="const", bufs=1) as constp,
            tc.tile_pool(name="work", bufs=1) as wp,
            tc.tile_pool(name="small", bufs=2) as sp,
            tc.tile_pool(name="ps_mm", bufs=2, space="PSUM") as ps_mm,
            tc.tile_pool(name="ps_s", bufs=2, space="PSUM") as ps_s,
            tc.tile_pool(name="ps_l1", bufs=2, space="PSUM") as ps_l1,
        ):
            wot = constp.tile([128, 8, D], bf16)
            onesb = constp.tile([128, 128], bf16)
            boc = constp.tile([128, 8], f32)
            g0c = constp.tile([128, 8], f32)
            be0c = constp.tile([128, 8], f32)
            g1c = constp.tile([128, 8], f32)
            be1c = constp.tile([128, 8], f32)
            epsc = constp.tile([128, 1], f32)
            x = wp.tile([128, 8, NC_], bf16)
            nc.sync.dma_start(x, XIN)
            nc.sync.dma_start(onesb, ONESB)
            nc.sync.dma_start(epsc, EPSC)
            nc.sync.dma_start(wot, WOT)
            nc.sync.dma_start(boc, BO)
            nc.sync.dma_start(g0c, G0)
            nc.sync.dma_start(be0c, BE0)
            nc.sync.dma_start(g1c, G1)
            nc.sync.dma_start(be1c, BE1)

            def ln_stats(x_in, sq_in, uid):
                """Broadcast mean/rstd [128, NC_] from psum-accumulated
                column sums. Short chain: var folded into the Sqrt activation
                (scale=1/D, bias=eps), mean cast fused with its 1/D scale."""
                ps1, ps2 = sq_in
                mb16 = sp.tile([128, 1, NC_], bf16, tag="mb16", name=f"mb16_{uid}")
                nc.scalar.activation(
                    mb16[:, 0, :], ps1, AF.Copy, scale=1.0 / D
                )
                v1 = sp.tile([128, NC_], f32, tag="v1", name=f"v1_{uid}")
                nc.vector.scalar_tensor_tensor(
                    out=v1,
                    in0=ps1,
                    scalar=1.0,
                    in1=mb16[:, 0, :],
                    op0=ALU.mult,
                    op1=ALU.mult,
                )
                dif = sp.tile([128, NC_], f32, tag="dif", name=f"dif_{uid}")
                nc.vector.tensor_tensor(dif, ps2, v1, ALU.subtract)
                sq = sp.tile([128, NC_], f32, tag="sq", name=f"sq_{uid}")
                nc.scalar.activation(
                    sq, dif, AF.Sqrt, scale=1.0 / D, bias=epsc
                )
                rstd = sp.tile([128, NC_], f32, tag="rstd", name=f"rstd_{uid}")
                nc.vector.reciprocal_approx_fast(rstd, sq)
                rb16 = sp.tile([128, 1, NC_], bf16, tag="rb16", name=f"rb16_{uid}")
                nc.scalar.activation(rb16[:, 0, :], rstd, AF.Copy)
                return mb16, rb16

            def ln_xhat(x_in, mb16, rb16, uid):
                """(x - mean) * rstd, per chunk (stride-0 broadcast DVE reads
                measured slower than per-chunk ops)."""
                xr = sp.tile([128, 8, NC_], bf16, tag="xra", name=f"xra_{uid}")
                for dc in range(8):
                    xc = sp.tile(
                        [128, NC_], bf16, tag="xca", name=f"xca_{uid}_{dc}"
                    )
                    nc.vector.tensor_tensor(
                        xc, x_in[:, dc, :], mb16[:, 0, :], ALU.subtract
                    )
                    nc.vector.tensor_tensor(
                        xr[:, dc, :], xc, rb16[:, 0, :], ALU.mult
                    )
                return xr

            # ---- LN0: stats then apply ----
            ps1 = ps_s.tile([128, NC_], f32, tag="s", name="ps1_ln0")
            for dc in range(8):
                nc.tensor.matmul(
                    ps1, onesb, x[:, dc, :], start=(dc == 0), stop=(dc == 7)
                )
            xsq = sp.tile([128, 8, NC_], bf16, tag="xsq", name="xsq_ln0")
            nc.vector.tensor_tensor(xsq, x, x, ALU.mult)
            ps2 = ps_s.tile([128, NC_], f32, tag="s", name="ps2_ln0")
            for dc in range(8):
                nc.tensor.matmul(
                    ps2, onesb, xsq[:, dc, :], start=(dc == 0), stop=(dc == 7)
                )
            mb16, rb16 = ln_stats(x, (ps1, ps2), "ln0")
            xhat0 = ln_xhat(x, mb16, rb16, "ln0")
            oln = wp.tile([128, 8, NC_], bf16)
            for dc in range(8):
                nc.scalar.activation(
                    oln[:, dc, :],
                    xhat0[:, dc, :],
                    AF.Identity,
                    scale=g0c[:, dc : dc + 1],
                    bias=be0c[:, dc : dc + 1],
                )

            # ---- fc_o with fused relu/residual/LN1-stat accumulation ----
            t1 = wp.tile([128, 8, NC_], bf16)
            o2 = wp.tile([128, 8, NC_], bf16)
            o2sq = wp.tile([128, 8, NC_], bf16)
            l1s1 = ps_l1.tile([128, NC_], f32, tag="l1", name="l1s1")
            l1s2 = ps_l1.tile([128, NC_], f32, tag="l1", name="l1s2")
            def stat_mms(do):
                # LN1 stat contribution for chunk do (emitted one iteration
                # late so the PE never waits on the relu->residual chain)
                nc.tensor.matmul(
                    l1s1,
                    onesb,
                    o2[:, do, :],
                    start=(do == 0),
                    stop=(do == 7),
                    skip_group_check=True,
                )
                nc.tensor.matmul(
                    l1s2,
                    onesb,
                    o2sq[:, do, :],
                    start=(do == 0),
                    stop=(do == 7),
                    skip_group_check=True,
                )

            for do in range(8):
                ps = ps_mm.tile([128, NC_], f32, tag="mm", name=f"mm{do}")
                for di in range(8):
                    nc.tensor.matmul(
                        ps,
                        wot[:, di, do * 128 : (do + 1) * 128],
                        oln[:, di, :],
                        start=(di == 0),
                        stop=(di == 7),
                    )
                nc.scalar.activation(
                    t1[:, do, :], ps, AF.Relu, bias=boc[:, do : do + 1]
                )
                nc.vector.tensor_tensor(
                    o2[:, do, :], t1[:, do, :], oln[:, do, :], ALU.add
                )
                nc.vector.tensor_tensor(
                    o2sq[:, do, :], o2[:, do, :], o2[:, do, :], ALU.mult
                )
                if do >= 1:
                    stat_mms(do - 1)
            stat_mms(7)

            # ---- LN1 (chunk-wise output DMA overlaps the apply) ----
            mb16b, rb16b = ln_stats(o2, (l1s1, l1s2), "ln1")
            xhat1 = ln_xhat(o2, mb16b, rb16b, "ln1")
            o3 = wp.tile([128, 8, NC_], f32)
            for dc in range(8):
                nc.scalar.activation(
                    o3[:, dc, :],
                    xhat1[:, dc, :],
                    AF.Identity,
                    scale=g1c[:, dc : dc + 1],
                    bias=be1c[:, dc : dc + 1],
                )
                nc.sync.dma_start(OUT2[:, dc, :], o3[:, dc, :])

    nc.compile()
    return nc


def _run(nc, in_maps, trace=False):
    from concourse.bass_utils import run_bass_kernel_spmd

    return run_bass_kernel_spmd(nc, in_maps, list(range(NCORES)), trace=trace)


def kernel(**inputs):
    trace = bool(int(__import__("os").environ.get("KERNEL_TRACE", "0")))
    f32 = np.float32
    bf16 = ml_dtypes.bfloat16
    f8 = ml_dtypes.float8_e4m3fn
    Q = np.ascontiguousarray(inputs["Q"], dtype=f32)
    K = np.ascontiguousarray(inputs["K"], dtype=f32)
    Wq, Wk, Wv, Wo = (np.asarray(inputs[k], f32) for k in ("Wq", "Wk", "Wv", "Wo"))
    bq, bk, bv, bo = (np.asarray(inputs[k], f32) for k in ("bq", "bk", "bv", "bo"))
    g0, be0, g1, be1 = (np.asarray(inputs[k], f32) for k in ("g0", "be0", "g1", "be1"))

    QT = np.ascontiguousarray(Q.transpose(0, 2, 1)).astype(bf16)
    KT8 = (
        np.ascontiguousarray(K.transpose(0, 2, 1)).reshape(B, 8, 128, N).astype(f8)
    )
    ones8 = np.full((128, 2, 128), OMEGA, dtype=f8)
    identb = np.eye(128, dtype=bf16)

    in_maps = []
    for h in range(H):
        hs = slice(h * DH, (h + 1) * DH)
        wqh = np.ascontiguousarray(
            Wq[:, hs].reshape(8, 128, 128).transpose(1, 0, 2)
        ).astype(bf16)
        wkh = np.ascontiguousarray(
            (Wk[:, hs] * 32.0).reshape(8, 128, 128).transpose(1, 0, 2)
        ).astype(f8)
        wvh = np.ascontiguousarray(
            (Wv[:, hs] * 32.0).reshape(8, 128, 128).transpose(1, 0, 2)
        ).astype(f8)
        in_maps.append(
            {
                "QT": QT,
                "KT8": KT8,
                "WQ": wqh,
                "WK": wkh,
                "WV": wvh,
                "BQ": bq[hs].reshape(128, 1).astype(f32),
                "BK": (bk[hs] * SCALE).reshape(128, 1).astype(f32),
                "BV": bv[hs].reshape(128, 1).astype(f32),
                "ONES8": ones8,
                "IDENTB": identb,
            }
        )

    r1 = _run(_build_d1(), in_maps, trace=trace)
    LAST_EXEC_NS["d1"] = r1.exec_time_ns

    # ---- host reshard: per-head [B, DH, N] f32 -> per-core [8, 128, 512] ----
    # core c <-> (batch c//2, n-half c%2); chunk dim = head index
    OTall = np.stack([r1.results[h]["OT"] for h in range(H)])  # [8, B, 128, N]
    wot_in = np.ascontiguousarray(
        Wo.reshape(8, 128, D).transpose(1, 0, 2)
    ).astype(bf16)
    onesb = np.ones((128, 128), dtype=bf16)
    col = lambda z: np.ascontiguousarray(z.reshape(8, 128).T, dtype=f32)
    boc, g0c, be0c, g1c, be1c = col(bo), col(g0), col(be0), col(g1), col(be1)
    in_maps2 = []
    for c in range(NCORES):
        b, nh = c // 2, c % 2
        # partition-major [128 p, 8 dc, 512 n] so the device DMA is contiguous
        xin = np.ascontiguousarray(
            OTall[:, b, :, nh * 512 : (nh + 1) * 512].transpose(1, 0, 2)
        ).astype(bf16)
        in_maps2.append(
            {
                "XIN": xin,
                "WOT": wot_in,
                "ONESB": onesb,
                "BO": boc,
                "G0": g0c,
                "BE0": be0c,
                "G1": g1c,
                "BE1": be1c,
                "EPSC": np.full((128, 1), LN_EPS, dtype=f32),
            }
        )
    r2 = _run(_build_d2(), in_maps2, trace=trace)
    LAST_EXEC_NS["d2"] = r2.exec_time_ns

    # ---- host unshard: [128 p, 8 dc, 512 n] -> [n, d] rows of O ----
    out = np.empty((B, N, D), dtype=f32)
    for c in range(NCORES):
        b, nh = c // 2, c % 2
        slab = r2.results[c]["OUT2"]  # [128, 8, 512] f32
        out[b, nh * 512 : (nh + 1) * 512, :] = (
            slab.transpose(2, 1, 0).reshape(512, D)
        )
    return out
